# revision 8
# baseline (speedup 1.0000x reference)
"""CrossAttention Trainium2 kernel, 8-core SPMD, single-pass float32r.

Sharding: core c -> (batch b = c//2, head-group g = c%2).  Each core computes
8 of the 16 heads for one batch: q/k/v projections restricted to its
inner-dim slice [g*512:(g+1)*512], full attention for those heads, and a
partial output projection (contraction over its 512 inner dims).  The host
sums the two partial outputs per batch and adds the bias.

Precision: every matmul runs on the PE in float32r (1 cycle/row, ~12-bit
mantissa).  The correctness budget (rel err 2e-2) is ~100x looser than the
~1e-4 this yields, so no hi/lo split passes are needed -- 3x less PE work
than the exact-fp32 split scheme.

Per-core dataflow (all intermediates SBUF-resident, no DRAM round-trips):
  vproj: v[m,512] -> va tiles [128, 8*65] (col h*65+64 is 1.0 so the
         softmax denominator rides the attn@v matmul)
  kproj: K^T head-pair tiles kt[p][128, M] (head 2p rows 0:64, 2p+1 64:128)
  qproj: Q^T head-pair tiles qt[p][128, N]  (scale folded into Wq)
  attn per (pair, jn-512-chunk): for each m-tile, two row-tiled K=64
    matmuls (array rows 0-63 / 64-127, concurrent) write simT into a
    2-bank PSUM tile; one wide exp (N=1024) -> fp32r e tile; two M=65
    attn@v matmuls accumulate [oT; denom] in per-head PSUM.  Normalize via
    reciprocal_approx_fast + gpsimd partition_broadcast + DVE mul, written
    straight into fp32r aoT tiles.
  oproj: out[n,1024] partial = aoT.T @ Wo, interleaved (like qproj) into
    the attention loop so PE work hides under the ACT-bound exp stream.
"""
import sys

sys.path.insert(0, "/opt/trn_rl_repo")

import numpy as np
import ml_dtypes

BF = ml_dtypes.bfloat16

import concourse.bacc as bacc
import concourse.mybir as mybir
import concourse.tile as tile
from concourse.bass_utils import run_bass_kernel_spmd

# bass_utils imports antenv.axon_hooks when trace=True; the read-only antenv
# package in this image lacks it, so register a no-op stub if missing.
try:
    import antenv.axon_hooks  # noqa: F401
except ImportError:
    import types as _types

    _stub = _types.ModuleType("antenv.axon_hooks")
    _stub.get_axon_ntff_profile_hook = lambda: None
    _stub.set_axon_ntff_profile_hook = lambda h: None
    sys.modules["antenv.axon_hooks"] = _stub

F32 = mybir.dt.float32
BF16 = mybir.dt.bfloat16
EXP = mybir.ActivationFunctionType.Exp

B, N, M = 4, 2048, 1024
QD, CD = 1024, 768
HEADS, DH = 16, 64
INNER = HEADS * DH
HG = 8            # heads per core
IS = HG * DH      # inner slice per core = 512
NC = 8

KQ = QD // 128    # 8  K-tiles for q projection
KC = CD // 128    # 6  K-tiles for k/v projection
NJ = N // 512     # 4  n chunks
MT = M // 128     # 8  m tiles
IT = IS // 128    # 4  inner tiles (= head pairs)

LAST_RESULTS = None  # stashed BassKernelResults for test.py introspection


def build_nc():
    nc = bacc.Bacc("TRN2", target_bir_lowering=False, debug=False, num_devices=NC)

    def din(name, shape):
        return nc.dram_tensor(name, shape, BF16, kind="ExternalInput").ap()

    xT = din("xT", [QD, N])
    cT = din("cT", [CD, M])
    wq = din("wq", [QD, IS])
    wk = din("wk", [CD, IS])
    wv = din("wv", [CD, IS])
    wo = din("wo", [IS, QD])
    out = nc.dram_tensor("out", [N, QD], F32, kind="ExternalOutput").ap()

    with tile.TileContext(nc) as tc:
        with tc.tile_pool(name="pers", bufs=1) as pers, \
             tc.tile_pool(name="wp", bufs=1) as wp, \
             tc.tile_pool(name="ps", bufs=1, space="PSUM") as psp, \
             tc.tile_pool(name="po", bufs=1, space="PSUM") as pop:

            qt = [pers.tile([128, N], BF16, tag=f"qt{p}", name=f"qt{p}")
                  for p in range(IT)]
            kt = [pers.tile([128, M], BF16, tag=f"kt{p}", name=f"kt{p}")
                  for p in range(IT)]
            va = [pers.tile([128, HG * 65], BF16, tag=f"va{mi}", name=f"va{mi}")
                  for mi in range(MT)]
            aot = [pers.tile([128, N], BF16, tag=f"aot{p}", name=f"aot{p}")
                   for p in range(IT)]
            wq_sb = [wp.tile([128, IS], BF16, tag=f"wq{k}", name=f"wq{k}")
                     for k in range(KQ)]
            wo_sb = [wp.tile([128, QD], BF16, tag=f"wo{k}", name=f"wo{k}")
                     for k in range(IT)]

            # ---------------- k/v projections (cT loaded once) -----------
            with nc.named_scope("kvproj"), \
                 tc.tile_pool(name="cp", bufs=1) as cp:
                ct_sb = [cp.tile([128, M], BF16, tag=f"ct{k}", name=f"ct{k}")
                         for k in range(KC)]
                wk_sb = [cp.tile([128, IS], BF16, tag=f"wk{k}", name=f"wk{k}")
                         for k in range(KC)]
                wv_sb = [cp.tile([128, IS], BF16, tag=f"wv{k}", name=f"wv{k}")
                         for k in range(KC)]
                for k in range(KC):
                    ksl = slice(k * 128, (k + 1) * 128)
                    nc.sync.dma_start(ct_sb[k][:], cT[ksl, :])
                    nc.sync.dma_start(wv_sb[k][:], wv[ksl, :])
                    nc.sync.dma_start(wk_sb[k][:], wk[ksl, :])
                for k in range(KQ):
                    ksl = slice(k * 128, (k + 1) * 128)
                    nc.sync.dma_start(wq_sb[k][:], wq[ksl, :])

                # vproj: out v[m-tile, inner 512] -> va (col h*65+64 = 1.0)
                onesf = cp.tile([128, HG], F32, tag="onesf", name="onesf")
                nc.vector.memset(onesf[:], 1.0)
                for mi in range(MT):
                    msl = slice(mi * 128, (mi + 1) * 128)
                    ps = psp.tile([128, 512], F32, tag="mm", name="mm", bufs=2)
                    for k in range(KC):
                        nc.tensor.matmul(ps[:], ct_sb[k][:, msl], wv_sb[k][:],
                                         start=(k == 0), stop=(k == KC - 1))
                    vcol = va[mi][:].rearrange("p (h c) -> p h c", c=65)
                    psv = ps[:].rearrange("p (h c) -> p h c", c=64)
                    nc.vector.tensor_copy(vcol[:, :, 0:64], psv[:])
                    nc.vector.tensor_copy(vcol[:, :, 64], onesf[:])

                # kproj: kt[p][:, msl] = Wk_p^T @ cT
                for p in range(IT):
                    for jm in range(M // 512):
                        msl = slice(jm * 512, (jm + 1) * 512)
                        isl = slice(p * 128, (p + 1) * 128)
                        ps = psp.tile([128, 512], F32, tag="mm", name="mm", bufs=2)
                        for k in range(KC):
                            nc.tensor.matmul(ps[:], wk_sb[k][:, isl],
                                             ct_sb[k][:, msl],
                                             start=(k == 0), stop=(k == KC - 1))
                        nc.vector.tensor_copy(kt[p][:, msl], ps[:])
                # load wo while attention runs
                for k in range(IT):
                    ksl = slice(k * 128, (k + 1) * 128)
                    nc.sync.dma_start(wo_sb[k][:], wo[ksl, :])

            # ---------------- qproj helper (per n-chunk) ----------------
            with tc.tile_pool(name="xs", bufs=2) as xsp, \
                 tc.tile_pool(name="sm", bufs=1) as smp, \
                 tc.tile_pool(name="ob", bufs=2) as obp:

                def qproj_load(jn):
                    nsl = slice(jn * 512, (jn + 1) * 512)
                    xs = [xsp.tile([128, 512], BF16, tag=f"x{k}", name=f"x{k}")
                          for k in range(KQ)]
                    for k in range(KQ):
                        ksl = slice(k * 128, (k + 1) * 128)
                        nc.sync.dma_start(xs[k][:], xT[ksl, nsl])
                    return xs

                def qproj_pair(jn, p, xs):
                    nsl = slice(jn * 512, (jn + 1) * 512)
                    isl = slice(p * 128, (p + 1) * 128)
                    ps = psp.tile([128, 512], F32, tag="mm", name="mm", bufs=2)
                    for k in range(KQ):
                        nc.tensor.matmul(ps[:], wq_sb[k][:, isl], xs[k][:],
                                         start=(k == 0), stop=(k == KQ - 1))
                    nc.vector.tensor_copy(qt[p][:, nsl], ps[:])

                def oproj_tile(nt):
                    tsl = slice(nt * 128, (nt + 1) * 128)
                    ob = obp.tile([128, QD], F32, tag="ob", name="ob")
                    for half in range(QD // 512):
                        qsl = slice(half * 512, (half + 1) * 512)
                        ps = psp.tile([128, 512], F32, tag="mm", name="mm",
                                      bufs=2)
                        for k in range(IT):
                            nc.tensor.matmul(ps[:], aot[k][:, tsl],
                                             wo_sb[k][:, qsl],
                                             start=(k == 0), stop=(k == IT - 1))
                        nc.vector.tensor_copy(ob[:, qsl], ps[:])
                    nc.sync.dma_start(out[tsl, :], ob[:])

                def attn_iter(hp, jn):
                    nsl = slice(jn * 512, (jn + 1) * 512)
                    he, ho = 2 * hp, 2 * hp + 1
                    po_e = pop.tile([65, 512], F32, tag="poe", name="poe")
                    po_o = pop.tile([65, 512], F32, tag="poo", name="poo")
                    for mi in range(MT):
                        msl = slice(mi * 128, (mi + 1) * 128)
                        ps = psp.tile([128, 1024], F32, tag="sp", name="sp",
                                      bufs=2)
                        nc.tensor.matmul(ps[:, 0:512], kt[hp][0:64, msl],
                                         qt[hp][0:64, nsl], start=True,
                                         stop=True)
                        nc.tensor.matmul(ps[:, 512:1024], kt[hp][64:128, msl],
                                         qt[hp][64:128, nsl], start=True,
                                         stop=True)
                        e = smp.tile([128, 1024], BF16, tag="e", name="e",
                                     bufs=4)
                        nc.scalar.activation(e[:], ps[:], EXP)
                        nc.tensor.matmul(po_e[:], va[mi][:, he * 65:he * 65 + 65],
                                         e[:, 0:512], start=(mi == 0),
                                         stop=(mi == MT - 1),
                                         skip_group_check=True)
                        nc.tensor.matmul(po_o[:], va[mi][:, ho * 65:ho * 65 + 65],
                                         e[:, 512:1024], start=(mi == 0),
                                         stop=(mi == MT - 1),
                                         skip_group_check=True)
                    for sub, po in ((0, po_e), (1, po_o)):
                        # recip_approx_fast NaNs on partition-offset inputs;
                        # stage the denominator row at partition 0 first.
                        dn = smp.tile([1, 512], F32, tag=f"dn{sub}",
                                      name=f"dn{sub}")
                        nc.vector.tensor_copy(dn[:], po[64:65, :])
                        rf = smp.tile([1, 512], F32, tag=f"rf{sub}",
                                      name=f"rf{sub}")
                        nc.vector.reciprocal_approx_fast(rf[:], dn[:])
                        pbs = smp.tile([64, 512], F32, tag=f"pbs{sub}",
                                       name=f"pbs{sub}")
                        nc.gpsimd.partition_broadcast(pbs[:], rf[:])
                        rsl = slice(sub * 64, sub * 64 + 64)
                        nc.vector.tensor_mul(aot[hp][rsl, nsl], po[0:64, :],
                                             pbs[:])

                # ---------------- main pipeline ----------------
                with nc.named_scope("attn"):
                    xs = qproj_load(0)
                    for p in range(IT):
                        qproj_pair(0, p, xs)
                    for jn in range(NJ):
                        if jn + 1 < NJ:
                            xs = qproj_load(jn + 1)
                        for hp in range(IT):
                            attn_iter(hp, jn)
                            if jn + 1 < NJ:
                                qproj_pair(jn + 1, hp, xs)
                            if jn >= 1:
                                oproj_tile(4 * (jn - 1) + hp)
                    for hp in range(IT):
                        oproj_tile(4 * (NJ - 1) + hp)
    nc.compile()
    return nc


_NC_CACHE = None


def kernel(x, context, Wq, Wk, Wv, Wo, bo, _trace=False):
    global _NC_CACHE, LAST_RESULTS
    x = np.asarray(x, np.float32)
    context = np.asarray(context, np.float32)
    scale = np.float32(DH ** -0.5)

    if _NC_CACHE is None:
        _NC_CACHE = build_nc()
    nc = _NC_CACHE

    Wq32 = np.asarray(Wq, np.float32)
    Wk32 = np.asarray(Wk, np.float32)
    Wv32 = np.asarray(Wv, np.float32)
    Wo32 = np.asarray(Wo, np.float32)
    in_maps = []
    for c in range(NC):
        b, g = c // 2, c % 2
        sl = slice(g * IS, (g + 1) * IS)
        m = {
            "xT": np.ascontiguousarray(x[b].T).astype(BF),
            "cT": np.ascontiguousarray(context[b].T).astype(BF),
            "wq": np.ascontiguousarray(Wq32[:, sl] * scale).astype(BF),
            "wk": np.ascontiguousarray(Wk32[:, sl]).astype(BF),
            "wv": np.ascontiguousarray(Wv32[:, sl]).astype(BF),
            "wo": np.ascontiguousarray(Wo32[sl, :]).astype(BF),
        }
        in_maps.append(m)
    res = run_bass_kernel_spmd(nc, in_maps, core_ids=list(range(NC)),
                               trace=_trace)
    LAST_RESULTS = res
    out = np.empty((B, N, QD), np.float32)
    bo32 = np.asarray(bo, np.float32)
    for b in range(B):
        out[b] = res.results[2 * b]["out"] + res.results[2 * b + 1]["out"] + bo32
    return out


# revision 9
# speedup vs baseline: 1.1214x; 1.1214x over previous
"""CrossAttention Trainium2 kernel, 8-core SPMD, single-pass float32r.

Sharding: core c -> (batch b = c//2, head-group g = c%2).  Each core computes
8 of the 16 heads for one batch: q/k/v projections restricted to its
inner-dim slice [g*512:(g+1)*512], full attention for those heads, and a
partial output projection (contraction over its 512 inner dims).  The host
sums the two partial outputs per batch and adds the bias.

Precision: every matmul runs on the PE in float32r (1 cycle/row, ~12-bit
mantissa).  The correctness budget (rel err 2e-2) is ~100x looser than the
~1e-4 this yields, so no hi/lo split passes are needed -- 3x less PE work
than the exact-fp32 split scheme.

Per-core dataflow (all intermediates SBUF-resident, no DRAM round-trips):
  vproj: v[m,512] -> va tiles [128, 8*65] (col h*65+64 is 1.0 so the
         softmax denominator rides the attn@v matmul)
  kproj: K^T head-pair tiles kt[p][128, M] (head 2p rows 0:64, 2p+1 64:128)
  qproj: Q^T head-pair tiles qt[p][128, N]  (scale folded into Wq)
  attn per (pair, jn-512-chunk): for each m-tile, two row-tiled K=64
    matmuls (array rows 0-63 / 64-127, concurrent) write simT into a
    2-bank PSUM tile; one wide exp (N=1024) -> fp32r e tile; two M=65
    attn@v matmuls accumulate [oT; denom] in per-head PSUM.  Normalize via
    reciprocal_approx_fast + gpsimd partition_broadcast + DVE mul, written
    straight into fp32r aoT tiles.
  oproj: out[n,1024] partial = aoT.T @ Wo, interleaved (like qproj) into
    the attention loop so PE work hides under the ACT-bound exp stream.
"""
import sys

sys.path.insert(0, "/opt/trn_rl_repo")

import numpy as np
import ml_dtypes

BF = ml_dtypes.bfloat16

import concourse.bacc as bacc
import concourse.mybir as mybir
import concourse.tile as tile
from concourse.bass_utils import run_bass_kernel_spmd

# bass_utils imports antenv.axon_hooks when trace=True; the read-only antenv
# package in this image lacks it, so register a no-op stub if missing.
try:
    import antenv.axon_hooks  # noqa: F401
except ImportError:
    import types as _types

    _stub = _types.ModuleType("antenv.axon_hooks")
    _stub.get_axon_ntff_profile_hook = lambda: None
    _stub.set_axon_ntff_profile_hook = lambda h: None
    sys.modules["antenv.axon_hooks"] = _stub

F32 = mybir.dt.float32
BF16 = mybir.dt.bfloat16
EXP = mybir.ActivationFunctionType.Exp

B, N, M = 4, 2048, 1024
QD, CD = 1024, 768
HEADS, DH = 16, 64
INNER = HEADS * DH
HG = 8            # heads per core
IS = HG * DH      # inner slice per core = 512
NC = 8

KQ = QD // 128    # 8  K-tiles for q projection
KC = CD // 128    # 6  K-tiles for k/v projection
NJ = N // 512     # 4  n chunks
MT = M // 128     # 8  m tiles
IT = IS // 128    # 4  inner tiles (= head pairs)

LAST_RESULTS = None  # stashed BassKernelResults for test.py introspection


def build_nc():
    nc = bacc.Bacc("TRN2", target_bir_lowering=False, debug=False, num_devices=NC)

    def din(name, shape):
        return nc.dram_tensor(name, shape, BF16, kind="ExternalInput").ap()

    xT = din("xT", [QD, N])
    cT = din("cT", [CD, M])
    wq = din("wq", [QD, IS])
    wk = din("wk", [CD, IS])
    wv = din("wv", [CD, IS])
    wo = din("wo", [IS, QD])
    out = nc.dram_tensor("out", [N, QD], F32, kind="ExternalOutput").ap()

    with tile.TileContext(nc) as tc:
        with tc.tile_pool(name="pers", bufs=1) as pers, \
             tc.tile_pool(name="wp", bufs=1) as wp, \
             tc.tile_pool(name="ps", bufs=1, space="PSUM") as psp, \
             tc.tile_pool(name="po", bufs=1, space="PSUM") as pop:

            qt = [pers.tile([128, N], BF16, tag=f"qt{p}", name=f"qt{p}")
                  for p in range(IT)]
            kt = [pers.tile([128, M], BF16, tag=f"kt{p}", name=f"kt{p}")
                  for p in range(IT)]
            va = [pers.tile([128, HG * 65], BF16, tag=f"va{mi}", name=f"va{mi}")
                  for mi in range(MT)]
            aot = [pers.tile([128, N], BF16, tag=f"aot{p}", name=f"aot{p}")
                   for p in range(IT)]
            wq_sb = [wp.tile([128, IS], BF16, tag=f"wq{k}", name=f"wq{k}")
                     for k in range(KQ)]
            wo_sb = [wp.tile([128, QD], BF16, tag=f"wo{k}", name=f"wo{k}")
                     for k in range(IT)]

            # ---------------- k/v projections (cT loaded once) -----------
            with nc.named_scope("kvproj"), \
                 tc.tile_pool(name="cp", bufs=1) as cp:
                ct_sb = [cp.tile([128, M], BF16, tag=f"ct{k}", name=f"ct{k}")
                         for k in range(KC)]
                wk_sb = [cp.tile([128, IS], BF16, tag=f"wk{k}", name=f"wk{k}")
                         for k in range(KC)]
                wv_sb = [cp.tile([128, IS], BF16, tag=f"wv{k}", name=f"wv{k}")
                         for k in range(KC)]
                for k in range(KC):
                    ksl = slice(k * 128, (k + 1) * 128)
                    nc.sync.dma_start(ct_sb[k][:], cT[ksl, :])
                    nc.sync.dma_start(wv_sb[k][:], wv[ksl, :])
                    nc.sync.dma_start(wk_sb[k][:], wk[ksl, :])
                for k in range(KQ):
                    ksl = slice(k * 128, (k + 1) * 128)
                    nc.sync.dma_start(wq_sb[k][:], wq[ksl, :])

                # vproj: out v[m-tile, inner 512] -> va (col h*65+64 = 1.0)
                onesf = cp.tile([128, HG], F32, tag="onesf", name="onesf")
                nc.vector.memset(onesf[:], 1.0)
                for mi in range(MT):
                    msl = slice(mi * 128, (mi + 1) * 128)
                    ps = psp.tile([128, 512], F32, tag="mm", name="mm", bufs=2)
                    for k in range(KC):
                        nc.tensor.matmul(ps[:], ct_sb[k][:, msl], wv_sb[k][:],
                                         start=(k == 0), stop=(k == KC - 1))
                    vcol = va[mi][:].rearrange("p (h c) -> p h c", c=65)
                    psv = ps[:].rearrange("p (h c) -> p h c", c=64)
                    nc.vector.tensor_copy(vcol[:, :, 0:64], psv[:])
                    nc.vector.tensor_copy(vcol[:, :, 64], onesf[:])

                # kproj: kt[p][:, msl] = Wk_p^T @ cT
                for p in range(IT):
                    for jm in range(M // 512):
                        msl = slice(jm * 512, (jm + 1) * 512)
                        isl = slice(p * 128, (p + 1) * 128)
                        ps = psp.tile([128, 512], F32, tag="mm", name="mm", bufs=2)
                        for k in range(KC):
                            nc.tensor.matmul(ps[:], wk_sb[k][:, isl],
                                             ct_sb[k][:, msl],
                                             start=(k == 0), stop=(k == KC - 1))
                        nc.vector.tensor_copy(kt[p][:, msl], ps[:])
                # load wo while attention runs
                for k in range(IT):
                    ksl = slice(k * 128, (k + 1) * 128)
                    nc.sync.dma_start(wo_sb[k][:], wo[ksl, :])

            # ---------------- qproj helper (per n-chunk) ----------------
            with tc.tile_pool(name="xs", bufs=2) as xsp, \
                 tc.tile_pool(name="sm", bufs=1) as smp, \
                 tc.tile_pool(name="ob", bufs=2) as obp:

                def qproj_load(jn):
                    nsl = slice(jn * 512, (jn + 1) * 512)
                    xs = [xsp.tile([128, 512], BF16, tag=f"x{k}", name=f"x{k}")
                          for k in range(KQ)]
                    for k in range(KQ):
                        ksl = slice(k * 128, (k + 1) * 128)
                        nc.sync.dma_start(xs[k][:], xT[ksl, nsl])
                    return xs

                def qproj_pair(jn, p, xs):
                    nsl = slice(jn * 512, (jn + 1) * 512)
                    isl = slice(p * 128, (p + 1) * 128)
                    ps = psp.tile([128, 512], F32, tag="mm", name="mm", bufs=2)
                    for k in range(KQ):
                        nc.tensor.matmul(ps[:], wq_sb[k][:, isl], xs[k][:],
                                         start=(k == 0), stop=(k == KQ - 1))
                    nc.vector.tensor_copy(qt[p][:, nsl], ps[:])

                def oproj_tile(nt):
                    tsl = slice(nt * 128, (nt + 1) * 128)
                    ob = obp.tile([128, QD], F32, tag="ob", name="ob")
                    for half in range(QD // 512):
                        qsl = slice(half * 512, (half + 1) * 512)
                        ps = psp.tile([128, 512], F32, tag="mm", name="mm",
                                      bufs=2)
                        for k in range(IT):
                            nc.tensor.matmul(ps[:], aot[k][:, tsl],
                                             wo_sb[k][:, qsl],
                                             start=(k == 0), stop=(k == IT - 1))
                        nc.vector.tensor_copy(ob[:, qsl], ps[:])
                    nc.sync.dma_start(out[tsl, :], ob[:])

                def attn_iter(hp, jn, fillers):
                    nsl = slice(jn * 512, (jn + 1) * 512)
                    he, ho = 2 * hp, 2 * hp + 1
                    po_e = pop.tile([65, 512], F32, tag="poe", name="poe")
                    po_o = pop.tile([65, 512], F32, tag="poo", name="poo")

                    def attnv(mi, e):
                        nc.tensor.matmul(po_e[:], va[mi][:, he * 65:he * 65 + 65],
                                         e[:, 0:512], start=(mi == 0),
                                         stop=(mi == MT - 1),
                                         skip_group_check=True)
                        nc.tensor.matmul(po_o[:], va[mi][:, ho * 65:ho * 65 + 65],
                                         e[:, 512:1024], start=(mi == 0),
                                         stop=(mi == MT - 1),
                                         skip_group_check=True)

                    # attnv runs one m-tile behind sim/exp so the PE never
                    # stalls on the ACT exp; filler matmul chains (qproj /
                    # oproj) slot in between to keep the PE queue fed.
                    pend = None
                    for mi in range(MT):
                        msl = slice(mi * 128, (mi + 1) * 128)
                        ps = psp.tile([128, 1024], F32, tag="sp", name="sp",
                                      bufs=2)
                        nc.tensor.matmul(ps[:, 0:512], kt[hp][0:64, msl],
                                         qt[hp][0:64, nsl], start=True,
                                         stop=True)
                        nc.tensor.matmul(ps[:, 512:1024], kt[hp][64:128, msl],
                                         qt[hp][64:128, nsl], start=True,
                                         stop=True)
                        e = smp.tile([128, 1024], BF16, tag="e", name="e",
                                     bufs=4)
                        nc.scalar.activation(e[:], ps[:], EXP)
                        if mi % 2 == 1 and fillers:
                            fillers.pop(0)()
                        if pend is not None:
                            attnv(*pend)
                        pend = (mi, e)
                    attnv(*pend)
                    for sub, po in ((0, po_e), (1, po_o)):
                        # recip_approx_fast NaNs on partition-offset inputs;
                        # stage the denominator row at partition 0 first.
                        dn = smp.tile([1, 512], F32, tag=f"dn{sub}",
                                      name=f"dn{sub}")
                        nc.vector.tensor_copy(dn[:], po[64:65, :])
                        rf = smp.tile([1, 512], F32, tag=f"rf{sub}",
                                      name=f"rf{sub}")
                        nc.vector.reciprocal_approx_fast(rf[:], dn[:])
                        pbs = smp.tile([64, 512], F32, tag=f"pbs{sub}",
                                       name=f"pbs{sub}")
                        nc.gpsimd.partition_broadcast(pbs[:], rf[:])
                        rsl = slice(sub * 64, sub * 64 + 64)
                        nc.vector.tensor_mul(aot[hp][rsl, nsl], po[0:64, :],
                                             pbs[:])

                # ---------------- main pipeline ----------------
                with nc.named_scope("attn"):
                    xs = qproj_load(0)
                    for p in range(IT):
                        qproj_pair(0, p, xs)
                    for jn in range(NJ):
                        fillers = []
                        if jn + 1 < NJ:
                            xs = qproj_load(jn + 1)
                            fillers += [
                                (lambda p=p, xs=xs, j=jn + 1: qproj_pair(j, p, xs))
                                for p in range(IT)]
                        if jn >= 1:
                            fillers += [
                                (lambda nt=nt: oproj_tile(nt))
                                for nt in range(4 * (jn - 1), 4 * jn)]
                        for hp in range(IT):
                            attn_iter(hp, jn, fillers)
                        for f in fillers:
                            f()
                    for hp in range(IT):
                        oproj_tile(4 * (NJ - 1) + hp)
    nc.compile()
    return nc


_NC_CACHE = None


def kernel(x, context, Wq, Wk, Wv, Wo, bo, _trace=False):
    global _NC_CACHE, LAST_RESULTS
    x = np.asarray(x, np.float32)
    context = np.asarray(context, np.float32)
    scale = np.float32(DH ** -0.5)

    if _NC_CACHE is None:
        _NC_CACHE = build_nc()
    nc = _NC_CACHE

    Wq32 = np.asarray(Wq, np.float32)
    Wk32 = np.asarray(Wk, np.float32)
    Wv32 = np.asarray(Wv, np.float32)
    Wo32 = np.asarray(Wo, np.float32)
    in_maps = []
    for c in range(NC):
        b, g = c // 2, c % 2
        sl = slice(g * IS, (g + 1) * IS)
        m = {
            "xT": np.ascontiguousarray(x[b].T).astype(BF),
            "cT": np.ascontiguousarray(context[b].T).astype(BF),
            "wq": np.ascontiguousarray(Wq32[:, sl] * scale).astype(BF),
            "wk": np.ascontiguousarray(Wk32[:, sl]).astype(BF),
            "wv": np.ascontiguousarray(Wv32[:, sl]).astype(BF),
            "wo": np.ascontiguousarray(Wo32[sl, :]).astype(BF),
        }
        in_maps.append(m)
    res = run_bass_kernel_spmd(nc, in_maps, core_ids=list(range(NC)),
                               trace=_trace)
    LAST_RESULTS = res
    out = np.empty((B, N, QD), np.float32)
    bo32 = np.asarray(bo, np.float32)
    for b in range(B):
        out[b] = res.results[2 * b]["out"] + res.results[2 * b + 1]["out"] + bo32
    return out


# revision 10
# speedup vs baseline: 1.2295x; 1.0963x over previous
"""CrossAttention Trainium2 kernel, 8-core SPMD, single-pass float32r.

Sharding: core c -> (batch b = c//2, head-group g = c%2).  Each core computes
8 of the 16 heads for one batch: q/k/v projections restricted to its
inner-dim slice [g*512:(g+1)*512], full attention for those heads, and a
partial output projection (contraction over its 512 inner dims).  The host
sums the two partial outputs per batch and adds the bias.

Precision: every matmul runs on the PE in float32r (1 cycle/row, ~12-bit
mantissa).  The correctness budget (rel err 2e-2) is ~100x looser than the
~1e-4 this yields, so no hi/lo split passes are needed -- 3x less PE work
than the exact-fp32 split scheme.

Per-core dataflow (all intermediates SBUF-resident, no DRAM round-trips):
  vproj: v[m,512] -> va tiles [128, 8*65] (col h*65+64 is 1.0 so the
         softmax denominator rides the attn@v matmul)
  kproj: K^T head-pair tiles kt[p][128, M] (head 2p rows 0:64, 2p+1 64:128)
  qproj: Q^T head-pair tiles qt[p][128, N]  (scale folded into Wq)
  attn per (pair, jn-512-chunk): for each m-tile, two row-tiled K=64
    matmuls (array rows 0-63 / 64-127, concurrent) write simT into a
    2-bank PSUM tile; one wide exp (N=1024) -> fp32r e tile; two M=65
    attn@v matmuls accumulate [oT; denom] in per-head PSUM.  Normalize via
    reciprocal_approx_fast + gpsimd partition_broadcast + DVE mul, written
    straight into fp32r aoT tiles.
  oproj: out[n,1024] partial = aoT.T @ Wo, interleaved (like qproj) into
    the attention loop so PE work hides under the ACT-bound exp stream.
"""
import sys

sys.path.insert(0, "/opt/trn_rl_repo")

import numpy as np
import ml_dtypes

BF = ml_dtypes.bfloat16

import concourse.bacc as bacc
import concourse.mybir as mybir
import concourse.tile as tile
from concourse.bass_utils import run_bass_kernel_spmd

# bass_utils imports antenv.axon_hooks when trace=True; the read-only antenv
# package in this image lacks it, so register a no-op stub if missing.
try:
    import antenv.axon_hooks  # noqa: F401
except ImportError:
    import types as _types

    _stub = _types.ModuleType("antenv.axon_hooks")
    _stub.get_axon_ntff_profile_hook = lambda: None
    _stub.set_axon_ntff_profile_hook = lambda h: None
    sys.modules["antenv.axon_hooks"] = _stub

F32 = mybir.dt.float32
BF16 = mybir.dt.bfloat16
EXP = mybir.ActivationFunctionType.Exp

B, N, M = 4, 2048, 1024
QD, CD = 1024, 768
HEADS, DH = 16, 64
INNER = HEADS * DH
HG = 8            # heads per core
IS = HG * DH      # inner slice per core = 512
NC = 8

KQ = QD // 128    # 8  K-tiles for q projection
KC = CD // 128    # 6  K-tiles for k/v projection
NJ = N // 512     # 4  n chunks
MT = M // 128     # 8  m tiles
IT = IS // 128    # 4  inner tiles (= head pairs)

LAST_RESULTS = None  # stashed BassKernelResults for test.py introspection


def build_nc():
    nc = bacc.Bacc("TRN2", target_bir_lowering=False, debug=False, num_devices=NC)

    def din(name, shape):
        return nc.dram_tensor(name, shape, BF16, kind="ExternalInput").ap()

    xT = din("xT", [QD, N])
    cT = din("cT", [CD, M])
    wq = din("wq", [QD, IS])
    wk = din("wk", [CD, IS])
    wv = din("wv", [CD, IS])
    wo = din("wo", [IS, QD])
    out = nc.dram_tensor("out", [N, QD], F32, kind="ExternalOutput").ap()

    with tile.TileContext(nc) as tc:
        with tc.tile_pool(name="pers", bufs=1) as pers, \
             tc.tile_pool(name="wp", bufs=1) as wp, \
             tc.tile_pool(name="ps", bufs=1, space="PSUM") as psp, \
             tc.tile_pool(name="po", bufs=1, space="PSUM") as pop:

            qt = [pers.tile([128, N], BF16, tag=f"qt{p}", name=f"qt{p}")
                  for p in range(IT)]
            kt = [pers.tile([128, M], BF16, tag=f"kt{p}", name=f"kt{p}")
                  for p in range(IT)]
            va = [pers.tile([128, HG * 65], BF16, tag=f"va{mi}", name=f"va{mi}")
                  for mi in range(MT)]
            aot = [pers.tile([128, N], BF16, tag=f"aot{p}", name=f"aot{p}")
                   for p in range(IT)]
            wq_sb = [wp.tile([128, IS], BF16, tag=f"wq{k}", name=f"wq{k}")
                     for k in range(KQ)]
            wo_sb = [wp.tile([128, QD], BF16, tag=f"wo{k}", name=f"wo{k}")
                     for k in range(IT)]

            # ---------------- k/v projections (cT loaded once) -----------
            with nc.named_scope("kvproj"):
                cp = wp
                ct_sb = [cp.tile([128, M], BF16, tag=f"ct{k}", name=f"ct{k}")
                         for k in range(KC)]
                wk_sb = [cp.tile([128, IS], BF16, tag=f"wk{k}", name=f"wk{k}")
                         for k in range(KC)]
                wv_sb = [cp.tile([128, IS], BF16, tag=f"wv{k}", name=f"wv{k}")
                         for k in range(KC)]
                for k in range(KC):
                    ksl = slice(k * 128, (k + 1) * 128)
                    nc.sync.dma_start(ct_sb[k][:], cT[ksl, :])
                    nc.sync.dma_start(wv_sb[k][:], wv[ksl, :])
                    nc.sync.dma_start(wk_sb[k][:], wk[ksl, :])
                for k in range(KQ):
                    ksl = slice(k * 128, (k + 1) * 128)
                    nc.sync.dma_start(wq_sb[k][:], wq[ksl, :])

                # vproj: out v[m-tile, inner 512] -> va (col h*65+64 = 1.0)
                onesf = cp.tile([128, HG], F32, tag="onesf", name="onesf")
                nc.vector.memset(onesf[:], 1.0)
                for mi in range(MT):
                    msl = slice(mi * 128, (mi + 1) * 128)
                    ps = psp.tile([128, 512], F32, tag="mm", name="mm", bufs=2)
                    for k in range(KC):
                        nc.tensor.matmul(ps[:], ct_sb[k][:, msl], wv_sb[k][:],
                                         start=(k == 0), stop=(k == KC - 1))
                    vcol = va[mi][:].rearrange("p (h c) -> p h c", c=65)
                    psv = ps[:].rearrange("p (h c) -> p h c", c=64)
                    nc.vector.tensor_copy(vcol[:, :, 0:64], psv[:])
                    nc.vector.tensor_copy(vcol[:, :, 64], onesf[:])

                # kproj chain for one (pair, m-half); pair 0 runs in the
                # prefix, pairs 1-3 become attn fillers
                def kproj_half(p, jm):
                    msl = slice(jm * 512, (jm + 1) * 512)
                    isl = slice(p * 128, (p + 1) * 128)
                    ps = psp.tile([128, 512], F32, tag="mm", name="mm", bufs=2)
                    for k in range(KC):
                        nc.tensor.matmul(ps[:], wk_sb[k][:, isl],
                                         ct_sb[k][:, msl],
                                         start=(k == 0), stop=(k == KC - 1))
                    nc.vector.tensor_copy(kt[p][:, msl], ps[:])

                for jm in range(M // 512):
                    kproj_half(0, jm)
                # load wo while attention runs
                for k in range(IT):
                    ksl = slice(k * 128, (k + 1) * 128)
                    nc.sync.dma_start(wo_sb[k][:], wo[ksl, :])

            # ---------------- qproj helper (per n-chunk) ----------------
            with tc.tile_pool(name="xs", bufs=2) as xsp, \
                 tc.tile_pool(name="sm", bufs=1) as smp, \
                 tc.tile_pool(name="ob", bufs=2) as obp:

                def qproj_load(jn):
                    nsl = slice(jn * 512, (jn + 1) * 512)
                    xs = [xsp.tile([128, 512], BF16, tag=f"x{k}", name=f"x{k}")
                          for k in range(KQ)]
                    for k in range(KQ):
                        ksl = slice(k * 128, (k + 1) * 128)
                        nc.sync.dma_start(xs[k][:], xT[ksl, nsl])
                    return xs

                def qproj_pair(jn, p, xs):
                    nsl = slice(jn * 512, (jn + 1) * 512)
                    isl = slice(p * 128, (p + 1) * 128)
                    ps = psp.tile([128, 512], F32, tag="mm", name="mm", bufs=2)
                    for k in range(KQ):
                        nc.tensor.matmul(ps[:], wq_sb[k][:, isl], xs[k][:],
                                         start=(k == 0), stop=(k == KQ - 1))
                    nc.vector.tensor_copy(qt[p][:, nsl], ps[:])

                def oproj_tile(nt):
                    tsl = slice(nt * 128, (nt + 1) * 128)
                    ob = obp.tile([128, QD], F32, tag="ob", name="ob")
                    for half in range(QD // 512):
                        qsl = slice(half * 512, (half + 1) * 512)
                        ps = psp.tile([128, 512], F32, tag="mm", name="mm",
                                      bufs=2)
                        for k in range(IT):
                            nc.tensor.matmul(ps[:], aot[k][:, tsl],
                                             wo_sb[k][:, qsl],
                                             start=(k == 0), stop=(k == IT - 1))
                        nc.vector.tensor_copy(ob[:, qsl], ps[:])
                    nc.sync.dma_start(out[tsl, :], ob[:])

                def attn_iter(hp, jn, fillers):
                    nsl = slice(jn * 512, (jn + 1) * 512)
                    he, ho = 2 * hp, 2 * hp + 1
                    po_e = pop.tile([65, 512], F32, tag="poe", name="poe")
                    po_o = pop.tile([65, 512], F32, tag="poo", name="poo")

                    def attnv(mi, e):
                        nc.tensor.matmul(po_e[:], va[mi][:, he * 65:he * 65 + 65],
                                         e[:, 0:512], start=(mi == 0),
                                         stop=(mi == MT - 1),
                                         skip_group_check=True)
                        nc.tensor.matmul(po_o[:], va[mi][:, ho * 65:ho * 65 + 65],
                                         e[:, 512:1024], start=(mi == 0),
                                         stop=(mi == MT - 1),
                                         skip_group_check=True)

                    # attnv runs one m-tile behind sim/exp so the PE never
                    # stalls on the ACT exp; filler matmul chains (qproj /
                    # oproj) slot in between to keep the PE queue fed.
                    pend = None
                    for mi in range(MT):
                        msl = slice(mi * 128, (mi + 1) * 128)
                        ps = psp.tile([128, 1024], F32, tag="sp", name="sp",
                                      bufs=2)
                        nc.tensor.matmul(ps[:, 0:512], kt[hp][0:64, msl],
                                         qt[hp][0:64, nsl], start=True,
                                         stop=True)
                        nc.tensor.matmul(ps[:, 512:1024], kt[hp][64:128, msl],
                                         qt[hp][64:128, nsl], start=True,
                                         stop=True)
                        e = smp.tile([128, 1024], BF16, tag="e", name="e",
                                     bufs=4)
                        nc.scalar.activation(e[:], ps[:], EXP)
                        if mi % 2 == 1 and fillers:
                            fillers.pop(0)()
                        if pend is not None:
                            attnv(*pend)
                        pend = (mi, e)
                    attnv(*pend)
                    for sub, po in ((0, po_e), (1, po_o)):
                        # recip_approx_fast NaNs on partition-offset inputs;
                        # stage the denominator row at partition 0 first.
                        dn = smp.tile([1, 512], F32, tag=f"dn{sub}",
                                      name=f"dn{sub}")
                        nc.vector.tensor_copy(dn[:], po[64:65, :])
                        rf = smp.tile([1, 512], F32, tag=f"rf{sub}",
                                      name=f"rf{sub}")
                        nc.vector.reciprocal_approx_fast(rf[:], dn[:])
                        pbs = smp.tile([64, 512], F32, tag=f"pbs{sub}",
                                       name=f"pbs{sub}")
                        nc.gpsimd.partition_broadcast(pbs[:], rf[:])
                        rsl = slice(sub * 64, sub * 64 + 64)
                        nc.vector.tensor_mul(aot[hp][rsl, nsl], po[0:64, :],
                                             pbs[:])

                # ---------------- main pipeline ----------------
                with nc.named_scope("attn"):
                    xs = qproj_load(0)
                    qproj_pair(0, 0, xs)
                    for jn in range(NJ):
                        fillers = []
                        if jn == 0:
                            xs0 = xs
                            for p in range(1, IT):
                                fillers += [
                                    (lambda p=p: kproj_half(p, 0)),
                                    (lambda p=p: kproj_half(p, 1)),
                                    (lambda p=p: qproj_pair(0, p, xs0))]
                        if jn + 1 < NJ:
                            xs = qproj_load(jn + 1)
                            fillers += [
                                (lambda p=p, xs=xs, j=jn + 1: qproj_pair(j, p, xs))
                                for p in range(IT)]
                        if jn >= 1:
                            fillers += [
                                (lambda nt=nt: oproj_tile(nt))
                                for nt in range(4 * (jn - 1), 4 * jn)]
                        for hp in range(IT):
                            attn_iter(hp, jn, fillers)
                        for f in fillers:
                            f()
                    for hp in range(IT):
                        oproj_tile(4 * (NJ - 1) + hp)
    nc.compile()
    return nc


_NC_CACHE = None


def kernel(x, context, Wq, Wk, Wv, Wo, bo, _trace=False):
    global _NC_CACHE, LAST_RESULTS
    x = np.asarray(x, np.float32)
    context = np.asarray(context, np.float32)
    scale = np.float32(DH ** -0.5)

    if _NC_CACHE is None:
        _NC_CACHE = build_nc()
    nc = _NC_CACHE

    Wq32 = np.asarray(Wq, np.float32)
    Wk32 = np.asarray(Wk, np.float32)
    Wv32 = np.asarray(Wv, np.float32)
    Wo32 = np.asarray(Wo, np.float32)
    in_maps = []
    for c in range(NC):
        b, g = c // 2, c % 2
        sl = slice(g * IS, (g + 1) * IS)
        m = {
            "xT": np.ascontiguousarray(x[b].T).astype(BF),
            "cT": np.ascontiguousarray(context[b].T).astype(BF),
            "wq": np.ascontiguousarray(Wq32[:, sl] * scale).astype(BF),
            "wk": np.ascontiguousarray(Wk32[:, sl]).astype(BF),
            "wv": np.ascontiguousarray(Wv32[:, sl]).astype(BF),
            "wo": np.ascontiguousarray(Wo32[sl, :]).astype(BF),
        }
        in_maps.append(m)
    res = run_bass_kernel_spmd(nc, in_maps, core_ids=list(range(NC)),
                               trace=_trace)
    LAST_RESULTS = res
    out = np.empty((B, N, QD), np.float32)
    bo32 = np.asarray(bo, np.float32)
    for b in range(B):
        out[b] = res.results[2 * b]["out"] + res.results[2 * b + 1]["out"] + bo32
    return out


# revision 12
# speedup vs baseline: 1.2499x; 1.0166x over previous
"""CrossAttention Trainium2 kernel, 8-core SPMD, single-pass float32r.

Sharding: core c -> (batch b = c//2, head-group g = c%2).  Each core computes
8 of the 16 heads for one batch: q/k/v projections restricted to its
inner-dim slice [g*512:(g+1)*512], full attention for those heads, and a
partial output projection (contraction over its 512 inner dims).  The host
sums the two partial outputs per batch and adds the bias.

Precision: every matmul runs on the PE in bf16 (1 cycle/row; separate
LDWEIGHTS instructions pipeline ahead of in-flight matmuls, unlike
fp32/fp32r whose 4-byte weight load serializes into the matmul), with fp32
PSUM accumulation.  Measured rel err ~3e-3 vs the 2e-2 budget.

Per-core dataflow (all intermediates SBUF-resident, no DRAM round-trips):
  vproj: v[m,512] -> va tiles [128, 8*65] (col h*65+64 is 1.0 so the
         softmax denominator rides the attn@v matmul)
  kproj: K^T head-pair tiles kt[p][128, M] (head 2p rows 0:64, 2p+1 64:128)
  qproj: Q^T head-pair tiles qt[p][128, N]  (scale folded into Wq)
  attn per (pair, jn-512-chunk): for each m-tile, two row-tiled K=64
    matmuls (array rows 0-63 / 64-127, concurrent) write simT into a
    2-bank PSUM tile; one wide exp (N=1024) -> bf16 e tile; two M=65
    attn@v matmuls accumulate [oT; denom] in per-head PSUM, software-
    pipelined one m-tile behind the exp so the PE never stalls on ACT.
    Normalize via reciprocal_approx_fast (input staged at partition 0 --
    the custom DVE op NaNs on partition-offset APs) + gpsimd
    partition_broadcast + DVE mul, written straight into bf16 aoT tiles.
  Only vproj + kproj(pair0) + qproj(jn0,pair0) run as a serial prefix;
  all remaining kproj/qproj chains and the oproj output tiles are fed as
  filler thunks between the attention matmuls, keeping the PE queue busy
  while the ACT engine streams the exp instructions.
"""
import sys

sys.path.insert(0, "/opt/trn_rl_repo")

import numpy as np
import ml_dtypes

BF = ml_dtypes.bfloat16

import concourse.bacc as bacc
import concourse.mybir as mybir
import concourse.tile as tile
from concourse.bass_utils import run_bass_kernel_spmd

# bass_utils imports antenv.axon_hooks when trace=True; the read-only antenv
# package in this image lacks it, so register a no-op stub if missing.
try:
    import antenv.axon_hooks  # noqa: F401
except ImportError:
    import types as _types

    _stub = _types.ModuleType("antenv.axon_hooks")
    _stub.get_axon_ntff_profile_hook = lambda: None
    _stub.set_axon_ntff_profile_hook = lambda h: None
    sys.modules["antenv.axon_hooks"] = _stub

F32 = mybir.dt.float32
BF16 = mybir.dt.bfloat16
EXP = mybir.ActivationFunctionType.Exp

B, N, M = 4, 2048, 1024
QD, CD = 1024, 768
HEADS, DH = 16, 64
INNER = HEADS * DH
HG = 8            # heads per core
IS = HG * DH      # inner slice per core = 512
NC = 8

KQ = QD // 128    # 8  K-tiles for q projection
KC = CD // 128    # 6  K-tiles for k/v projection
NJ = N // 512     # 4  n chunks
MT = M // 128     # 8  m tiles
IT = IS // 128    # 4  inner tiles (= head pairs)

LAST_RESULTS = None  # stashed BassKernelResults for test.py introspection


def build_nc():
    nc = bacc.Bacc("TRN2", target_bir_lowering=False, debug=False, num_devices=NC)

    def din(name, shape):
        return nc.dram_tensor(name, shape, BF16, kind="ExternalInput").ap()

    xT = din("xT", [QD, N])
    cT = din("cT", [CD, M])
    wq = din("wq", [QD, IS])
    wk = din("wk", [CD, IS])
    wv = din("wv", [CD, IS])
    wo = din("wo", [IS, QD])
    out = nc.dram_tensor("out", [N, QD], F32, kind="ExternalOutput").ap()

    with tile.TileContext(nc) as tc:
        with tc.tile_pool(name="pers", bufs=1) as pers, \
             tc.tile_pool(name="wp", bufs=1) as wp, \
             tc.tile_pool(name="ps", bufs=1, space="PSUM") as psp, \
             tc.tile_pool(name="po", bufs=1, space="PSUM") as pop:

            qt = [pers.tile([128, N], BF16, tag=f"qt{p}", name=f"qt{p}")
                  for p in range(IT)]
            kt = [pers.tile([128, M], BF16, tag=f"kt{p}", name=f"kt{p}")
                  for p in range(IT)]
            va = [pers.tile([128, HG * 65], BF16, tag=f"va{mi}", name=f"va{mi}")
                  for mi in range(MT)]
            aot = [pers.tile([128, N], BF16, tag=f"aot{p}", name=f"aot{p}")
                   for p in range(IT)]
            wq_sb = [wp.tile([128, IS], BF16, tag=f"wq{k}", name=f"wq{k}")
                     for k in range(KQ)]
            wo_sb = [wp.tile([128, QD], BF16, tag=f"wo{k}", name=f"wo{k}")
                     for k in range(IT)]

            # ---------------- k/v projections (cT loaded once) -----------
            with nc.named_scope("kvproj"):
                cp = wp
                ct_sb = [cp.tile([128, M], BF16, tag=f"ct{k}", name=f"ct{k}")
                         for k in range(KC)]
                wk_sb = [cp.tile([128, IS], BF16, tag=f"wk{k}", name=f"wk{k}")
                         for k in range(KC)]
                wv_sb = [cp.tile([128, IS], BF16, tag=f"wv{k}", name=f"wv{k}")
                         for k in range(KC)]
                for k in range(KC):
                    ksl = slice(k * 128, (k + 1) * 128)
                    nc.sync.dma_start(wv_sb[k][:], wv[ksl, :])
                for k in range(KC):
                    ksl = slice(k * 128, (k + 1) * 128)
                    nc.sync.dma_start(ct_sb[k][:, 0:256], cT[ksl, 0:256])
                for k in range(KC):
                    ksl = slice(k * 128, (k + 1) * 128)
                    nc.sync.dma_start(ct_sb[k][:, 256:M], cT[ksl, 256:M])
                for k in range(KC):
                    ksl = slice(k * 128, (k + 1) * 128)
                    nc.sync.dma_start(wk_sb[k][:], wk[ksl, :])
                for k in range(KQ):
                    ksl = slice(k * 128, (k + 1) * 128)
                    nc.sync.dma_start(wq_sb[k][:], wq[ksl, :])

                # vproj: out v[m-tile, inner 512] -> va (col h*65+64 = 1.0)
                onesf = cp.tile([128, HG], F32, tag="onesf", name="onesf")
                nc.vector.memset(onesf[:], 1.0)
                for mi in range(MT):
                    msl = slice(mi * 128, (mi + 1) * 128)
                    ps = psp.tile([128, 512], F32, tag="mm", name="mm", bufs=2)
                    for k in range(KC):
                        nc.tensor.matmul(ps[:], ct_sb[k][:, msl], wv_sb[k][:],
                                         start=(k == 0), stop=(k == KC - 1))
                    vcol = va[mi][:].rearrange("p (h c) -> p h c", c=65)
                    psv = ps[:].rearrange("p (h c) -> p h c", c=64)
                    nc.vector.tensor_copy(vcol[:, :, 0:64], psv[:])
                    nc.vector.tensor_copy(vcol[:, :, 64], onesf[:])

                # kproj chain for one (pair, m-half); pair 0 runs in the
                # prefix, pairs 1-3 become attn fillers
                def kproj_half(p, jm):
                    msl = slice(jm * 512, (jm + 1) * 512)
                    isl = slice(p * 128, (p + 1) * 128)
                    ps = psp.tile([128, 512], F32, tag="mm", name="mm", bufs=2)
                    for k in range(KC):
                        nc.tensor.matmul(ps[:], wk_sb[k][:, isl],
                                         ct_sb[k][:, msl],
                                         start=(k == 0), stop=(k == KC - 1))
                    nc.vector.tensor_copy(kt[p][:, msl], ps[:])

                for jm in range(M // 512):
                    kproj_half(0, jm)
                # load wo while attention runs
                for k in range(IT):
                    ksl = slice(k * 128, (k + 1) * 128)
                    nc.sync.dma_start(wo_sb[k][:], wo[ksl, :])

            # ---------------- qproj helper (per n-chunk) ----------------
            with tc.tile_pool(name="xs", bufs=2) as xsp, \
                 tc.tile_pool(name="sm", bufs=1) as smp, \
                 tc.tile_pool(name="ob", bufs=2) as obp:

                def qproj_load(jn):
                    nsl = slice(jn * 512, (jn + 1) * 512)
                    xs = [xsp.tile([128, 512], BF16, tag=f"x{k}", name=f"x{k}")
                          for k in range(KQ)]
                    for k in range(KQ):
                        ksl = slice(k * 128, (k + 1) * 128)
                        nc.sync.dma_start(xs[k][:], xT[ksl, nsl])
                    return xs

                def qproj_pair(jn, p, xs):
                    nsl = slice(jn * 512, (jn + 1) * 512)
                    isl = slice(p * 128, (p + 1) * 128)
                    ps = psp.tile([128, 512], F32, tag="mm", name="mm", bufs=2)
                    for k in range(KQ):
                        nc.tensor.matmul(ps[:], wq_sb[k][:, isl], xs[k][:],
                                         start=(k == 0), stop=(k == KQ - 1))
                    nc.vector.tensor_copy(qt[p][:, nsl], ps[:])

                def oproj_tile(nt):
                    tsl = slice(nt * 128, (nt + 1) * 128)
                    ob = obp.tile([128, QD], F32, tag="ob", name="ob")
                    for half in range(QD // 512):
                        qsl = slice(half * 512, (half + 1) * 512)
                        ps = psp.tile([128, 512], F32, tag="mm", name="mm",
                                      bufs=2)
                        for k in range(IT):
                            nc.tensor.matmul(ps[:], aot[k][:, tsl],
                                             wo_sb[k][:, qsl],
                                             start=(k == 0), stop=(k == IT - 1))
                        nc.vector.tensor_copy(ob[:, qsl], ps[:])
                    nc.sync.dma_start(out[tsl, :], ob[:])

                def attn_iter(hp, jn, fillers, last_jn=False):
                    nsl = slice(jn * 512, (jn + 1) * 512)
                    he, ho = 2 * hp, 2 * hp + 1
                    po_e = pop.tile([65, 512], F32, tag="poe", name="poe")
                    po_o = pop.tile([65, 512], F32, tag="poo", name="poo")

                    def attnv(mi, e):
                        nc.tensor.matmul(po_e[:], va[mi][:, he * 65:he * 65 + 65],
                                         e[:, 0:512], start=(mi == 0),
                                         stop=(mi == MT - 1),
                                         skip_group_check=True)
                        nc.tensor.matmul(po_o[:], va[mi][:, ho * 65:ho * 65 + 65],
                                         e[:, 512:1024], start=(mi == 0),
                                         stop=(mi == MT - 1),
                                         skip_group_check=True)

                    # attnv runs one m-tile behind sim/exp so the PE never
                    # stalls on the ACT exp; filler matmul chains (qproj /
                    # oproj) slot in between to keep the PE queue fed.
                    pend = None
                    for mi in range(MT):
                        msl = slice(mi * 128, (mi + 1) * 128)
                        ps = psp.tile([128, 1024], F32, tag="sp", name="sp",
                                      bufs=2)
                        nc.tensor.matmul(ps[:, 0:512], kt[hp][0:64, msl],
                                         qt[hp][0:64, nsl], start=True,
                                         stop=True)
                        nc.tensor.matmul(ps[:, 512:1024], kt[hp][64:128, msl],
                                         qt[hp][64:128, nsl], start=True,
                                         stop=True)
                        e = smp.tile([128, 1024], BF16, tag="e", name="e",
                                     bufs=4)
                        nc.scalar.activation(e[:], ps[:], EXP)
                        if fillers and (mi % 2 == 1 if not last_jn
                                        else mi == 3):
                            fillers.pop(0)()
                        if pend is not None:
                            attnv(*pend)
                        pend = (mi, e)
                    attnv(*pend)
                    for sub, po in ((0, po_e), (1, po_o)):
                        # recip_approx_fast NaNs on partition-offset inputs;
                        # stage the denominator row at partition 0 first.
                        dn = smp.tile([1, 512], F32, tag=f"dn{sub}",
                                      name=f"dn{sub}")
                        nc.vector.tensor_copy(dn[:], po[64:65, :])
                        rf = smp.tile([1, 512], F32, tag=f"rf{sub}",
                                      name=f"rf{sub}")
                        nc.vector.reciprocal_approx_fast(rf[:], dn[:])
                        pbs = smp.tile([64, 512], F32, tag=f"pbs{sub}",
                                       name=f"pbs{sub}")
                        nc.gpsimd.partition_broadcast(pbs[:], rf[:])
                        rsl = slice(sub * 64, sub * 64 + 64)
                        nc.vector.tensor_mul(aot[hp][rsl, nsl], po[0:64, :],
                                             pbs[:])

                # ---------------- main pipeline ----------------
                with nc.named_scope("attn"):
                    xs = qproj_load(0)
                    qproj_pair(0, 0, xs)
                    for jn in range(NJ):
                        fillers = []
                        if jn == 0:
                            xs0 = xs
                            for p in range(1, IT):
                                fillers += [
                                    (lambda p=p: kproj_half(p, 0)),
                                    (lambda p=p: kproj_half(p, 1)),
                                    (lambda p=p: qproj_pair(0, p, xs0))]
                        if jn + 1 < NJ:
                            xs = qproj_load(jn + 1)
                            fillers += [
                                (lambda p=p, xs=xs, j=jn + 1: qproj_pair(j, p, xs))
                                for p in range(IT)]
                        if jn >= 1:
                            fillers += [
                                (lambda nt=nt: oproj_tile(nt))
                                for nt in range(4 * (jn - 1), 4 * jn)]
                        for hp in range(IT):
                            attn_iter(hp, jn, fillers, jn == NJ - 1)
                        for f in fillers:
                            f()
                    for hp in range(IT):
                        oproj_tile(4 * (NJ - 1) + hp)
    nc.compile()
    return nc


_NC_CACHE = None


def kernel(x, context, Wq, Wk, Wv, Wo, bo, _trace=False):
    global _NC_CACHE, LAST_RESULTS
    x = np.asarray(x, np.float32)
    context = np.asarray(context, np.float32)
    scale = np.float32(DH ** -0.5)

    if _NC_CACHE is None:
        _NC_CACHE = build_nc()
    nc = _NC_CACHE

    Wq32 = np.asarray(Wq, np.float32)
    Wk32 = np.asarray(Wk, np.float32)
    Wv32 = np.asarray(Wv, np.float32)
    Wo32 = np.asarray(Wo, np.float32)
    in_maps = []
    for c in range(NC):
        b, g = c // 2, c % 2
        sl = slice(g * IS, (g + 1) * IS)
        m = {
            "xT": np.ascontiguousarray(x[b].T).astype(BF),
            "cT": np.ascontiguousarray(context[b].T).astype(BF),
            "wq": np.ascontiguousarray(Wq32[:, sl] * scale).astype(BF),
            "wk": np.ascontiguousarray(Wk32[:, sl]).astype(BF),
            "wv": np.ascontiguousarray(Wv32[:, sl]).astype(BF),
            "wo": np.ascontiguousarray(Wo32[sl, :]).astype(BF),
        }
        in_maps.append(m)
    res = run_bass_kernel_spmd(nc, in_maps, core_ids=list(range(NC)),
                               trace=_trace)
    LAST_RESULTS = res
    out = np.empty((B, N, QD), np.float32)
    bo32 = np.asarray(bo, np.float32)
    for b in range(B):
        out[b] = res.results[2 * b]["out"] + res.results[2 * b + 1]["out"] + bo32
    return out


# revision 13
# speedup vs baseline: 1.2638x; 1.0111x over previous
"""CrossAttention Trainium2 kernel, 8-core SPMD, single-pass float32r.

Sharding: core c -> (batch b = c//2, head-group g = c%2).  Each core computes
8 of the 16 heads for one batch: q/k/v projections restricted to its
inner-dim slice [g*512:(g+1)*512], full attention for those heads, and a
partial output projection (contraction over its 512 inner dims).  The host
sums the two partial outputs per batch and adds the bias.

Precision: every matmul runs on the PE in bf16 (1 cycle/row; separate
LDWEIGHTS instructions pipeline ahead of in-flight matmuls, unlike
fp32/fp32r whose 4-byte weight load serializes into the matmul), with fp32
PSUM accumulation.  Measured rel err ~3e-3 vs the 2e-2 budget.

Per-core dataflow (all intermediates SBUF-resident, no DRAM round-trips):
  vproj: v[m,512] -> va tiles [128, 8*65] (col h*65+64 is 1.0 so the
         softmax denominator rides the attn@v matmul)
  kproj: K^T head-pair tiles kt[p][128, M] (head 2p rows 0:64, 2p+1 64:128)
  qproj: Q^T head-pair tiles qt[p][128, N]  (scale folded into Wq)
  attn per (pair, jn-512-chunk): for each m-tile, two row-tiled K=64
    matmuls (array rows 0-63 / 64-127, concurrent) write simT into a
    2-bank PSUM tile; one wide exp (N=1024) -> bf16 e tile; two M=65
    attn@v matmuls accumulate [oT; denom] in per-head PSUM, software-
    pipelined one m-tile behind the exp so the PE never stalls on ACT.
    Normalize via reciprocal_approx_fast (input staged at partition 0 --
    the custom DVE op NaNs on partition-offset APs) + gpsimd
    partition_broadcast + DVE mul, written straight into bf16 aoT tiles.
  Only vproj + kproj(pair0) + qproj(jn0,pair0) run as a serial prefix;
  all remaining kproj/qproj chains and the oproj output tiles are fed as
  filler thunks between the attention matmuls, keeping the PE queue busy
  while the ACT engine streams the exp instructions.
"""
import sys

sys.path.insert(0, "/opt/trn_rl_repo")

import numpy as np
import ml_dtypes

BF = ml_dtypes.bfloat16

import concourse.bacc as bacc
import concourse.mybir as mybir
import concourse.tile as tile
from concourse.bass_utils import run_bass_kernel_spmd

# bass_utils imports antenv.axon_hooks when trace=True; the read-only antenv
# package in this image lacks it, so register a no-op stub if missing.
try:
    import antenv.axon_hooks  # noqa: F401
except ImportError:
    import types as _types

    _stub = _types.ModuleType("antenv.axon_hooks")
    _stub.get_axon_ntff_profile_hook = lambda: None
    _stub.set_axon_ntff_profile_hook = lambda h: None
    sys.modules["antenv.axon_hooks"] = _stub

F32 = mybir.dt.float32
BF16 = mybir.dt.bfloat16
EXP = mybir.ActivationFunctionType.Exp

B, N, M = 4, 2048, 1024
QD, CD = 1024, 768
HEADS, DH = 16, 64
INNER = HEADS * DH
HG = 8            # heads per core
IS = HG * DH      # inner slice per core = 512
NC = 8

KQ = QD // 128    # 8  K-tiles for q projection
KC = CD // 128    # 6  K-tiles for k/v projection
NJ = N // 512     # 4  n chunks
MT = M // 128     # 8  m tiles
IT = IS // 128    # 4  inner tiles (= head pairs)

LAST_RESULTS = None  # stashed BassKernelResults for test.py introspection


def build_nc():
    nc = bacc.Bacc("TRN2", target_bir_lowering=False, debug=False, num_devices=NC)

    def din(name, shape):
        return nc.dram_tensor(name, shape, BF16, kind="ExternalInput").ap()

    xT = din("xT", [QD, N])
    cT = din("cT", [CD, M])
    wq = din("wq", [QD, IS])
    wk = din("wk", [CD, IS])
    wv = din("wv", [CD, IS])
    wo = din("wo", [IS, QD])
    out = nc.dram_tensor("out", [N, QD], F32, kind="ExternalOutput").ap()

    with tile.TileContext(nc) as tc:
        with tc.tile_pool(name="pers", bufs=1) as pers, \
             tc.tile_pool(name="wp", bufs=1) as wp, \
             tc.tile_pool(name="ps", bufs=1, space="PSUM") as psp, \
             tc.tile_pool(name="po", bufs=1, space="PSUM") as pop:

            qt = [pers.tile([128, N], BF16, tag=f"qt{p}", name=f"qt{p}")
                  for p in range(IT)]
            kt = [pers.tile([128, M], BF16, tag=f"kt{p}", name=f"kt{p}")
                  for p in range(IT)]
            va = [pers.tile([128, HG * 65], BF16, tag=f"va{mi}", name=f"va{mi}")
                  for mi in range(MT)]
            aot = [pers.tile([128, N], BF16, tag=f"aot{p}", name=f"aot{p}")
                   for p in range(IT)]
            wq_sb = [wp.tile([128, IS], BF16, tag=f"wq{k}", name=f"wq{k}")
                     for k in range(KQ)]
            wo_sb = [wp.tile([128, QD], BF16, tag=f"wo{k}", name=f"wo{k}")
                     for k in range(IT)]

            # ---------------- k/v projections (cT loaded once) -----------
            with nc.named_scope("kvproj"):
                cp = wp
                ct_sb = [cp.tile([128, M], BF16, tag=f"ct{k}", name=f"ct{k}")
                         for k in range(KC)]
                wk_sb = [cp.tile([128, IS], BF16, tag=f"wk{k}", name=f"wk{k}")
                         for k in range(KC)]
                wv_sb = [cp.tile([128, IS], BF16, tag=f"wv{k}", name=f"wv{k}")
                         for k in range(KC)]
                for k in range(KC):
                    ksl = slice(k * 128, (k + 1) * 128)
                    nc.sync.dma_start(wv_sb[k][:], wv[ksl, :])
                for k in range(KC):
                    ksl = slice(k * 128, (k + 1) * 128)
                    nc.sync.dma_start(ct_sb[k][:, 0:512], cT[ksl, 0:512])
                for k in range(KC):
                    ksl = slice(k * 128, (k + 1) * 128)
                    nc.sync.dma_start(wk_sb[k][:], wk[ksl, :])
                for k in range(KC):
                    ksl = slice(k * 128, (k + 1) * 128)
                    nc.sync.dma_start(ct_sb[k][:, 512:M], cT[ksl, 512:M])
                for k in range(KQ):
                    ksl = slice(k * 128, (k + 1) * 128)
                    nc.sync.dma_start(wq_sb[k][:], wq[ksl, :])

                # vproj: out v[m-tile, inner 512] -> va (col h*65+64 = 1.0)
                onesf = cp.tile([128, HG], F32, tag="onesf", name="onesf")
                nc.vector.memset(onesf[:], 1.0)
                for mi in range(MT):
                    msl = slice(mi * 128, (mi + 1) * 128)
                    ps = psp.tile([128, 512], F32, tag="mm", name="mm", bufs=2)
                    for k in range(KC):
                        nc.tensor.matmul(ps[:], ct_sb[k][:, msl], wv_sb[k][:],
                                         start=(k == 0), stop=(k == KC - 1))
                    vcol = va[mi][:].rearrange("p (h c) -> p h c", c=65)
                    psv = ps[:].rearrange("p (h c) -> p h c", c=64)
                    nc.vector.tensor_copy(vcol[:, :, 0:64], psv[:])
                    nc.vector.tensor_copy(vcol[:, :, 64], onesf[:])

                # kproj chain for one (pair, m-half); pair 0 runs in the
                # prefix, pairs 1-3 become attn fillers
                def kproj_half(p, jm):
                    msl = slice(jm * 512, (jm + 1) * 512)
                    isl = slice(p * 128, (p + 1) * 128)
                    ps = psp.tile([128, 512], F32, tag="mm", name="mm", bufs=2)
                    for k in range(KC):
                        nc.tensor.matmul(ps[:], wk_sb[k][:, isl],
                                         ct_sb[k][:, msl],
                                         start=(k == 0), stop=(k == KC - 1))
                    nc.vector.tensor_copy(kt[p][:, msl], ps[:])

                for jm in range(M // 512):
                    kproj_half(0, jm)
                # load wo while attention runs
                for k in range(IT):
                    ksl = slice(k * 128, (k + 1) * 128)
                    nc.sync.dma_start(wo_sb[k][:], wo[ksl, :])

            # ---------------- qproj helper (per n-chunk) ----------------
            with tc.tile_pool(name="xs", bufs=2) as xsp, \
                 tc.tile_pool(name="sm", bufs=1) as smp, \
                 tc.tile_pool(name="ob", bufs=2) as obp:

                def qproj_load(jn):
                    nsl = slice(jn * 512, (jn + 1) * 512)
                    xs = [xsp.tile([128, 512], BF16, tag=f"x{k}", name=f"x{k}")
                          for k in range(KQ)]
                    for k in range(KQ):
                        ksl = slice(k * 128, (k + 1) * 128)
                        nc.sync.dma_start(xs[k][:], xT[ksl, nsl])
                    return xs

                def qproj_pair(jn, p, xs):
                    nsl = slice(jn * 512, (jn + 1) * 512)
                    isl = slice(p * 128, (p + 1) * 128)
                    ps = psp.tile([128, 512], F32, tag="mm", name="mm", bufs=2)
                    for k in range(KQ):
                        nc.tensor.matmul(ps[:], wq_sb[k][:, isl], xs[k][:],
                                         start=(k == 0), stop=(k == KQ - 1))
                    nc.vector.tensor_copy(qt[p][:, nsl], ps[:])

                def oproj_tile(nt):
                    tsl = slice(nt * 128, (nt + 1) * 128)
                    ob = obp.tile([128, QD], F32, tag="ob", name="ob")
                    for half in range(QD // 512):
                        qsl = slice(half * 512, (half + 1) * 512)
                        ps = psp.tile([128, 512], F32, tag="mm", name="mm",
                                      bufs=2)
                        for k in range(IT):
                            nc.tensor.matmul(ps[:], aot[k][:, tsl],
                                             wo_sb[k][:, qsl],
                                             start=(k == 0), stop=(k == IT - 1))
                        nc.vector.tensor_copy(ob[:, qsl], ps[:])
                    nc.sync.dma_start(out[tsl, :], ob[:])

                def attn_iter(hp, jn, fillers, last_jn=False):
                    nsl = slice(jn * 512, (jn + 1) * 512)
                    he, ho = 2 * hp, 2 * hp + 1
                    po_e = pop.tile([65, 512], F32, tag="poe", name="poe")
                    po_o = pop.tile([65, 512], F32, tag="poo", name="poo")

                    def attnv(mi, e):
                        nc.tensor.matmul(po_e[:], va[mi][:, he * 65:he * 65 + 65],
                                         e[:, 0:512], start=(mi == 0),
                                         stop=(mi == MT - 1),
                                         skip_group_check=True)
                        nc.tensor.matmul(po_o[:], va[mi][:, ho * 65:ho * 65 + 65],
                                         e[:, 512:1024], start=(mi == 0),
                                         stop=(mi == MT - 1),
                                         skip_group_check=True)

                    # attnv runs one m-tile behind sim/exp so the PE never
                    # stalls on the ACT exp; filler matmul chains (qproj /
                    # oproj) slot in between to keep the PE queue fed.
                    pend = None
                    for mi in range(MT):
                        msl = slice(mi * 128, (mi + 1) * 128)
                        ps = psp.tile([128, 1024], F32, tag="sp", name="sp",
                                      bufs=2)
                        nc.tensor.matmul(ps[:, 0:512], kt[hp][0:64, msl],
                                         qt[hp][0:64, nsl], start=True,
                                         stop=True)
                        nc.tensor.matmul(ps[:, 512:1024], kt[hp][64:128, msl],
                                         qt[hp][64:128, nsl], start=True,
                                         stop=True)
                        e = smp.tile([128, 1024], BF16, tag="e", name="e",
                                     bufs=4)
                        nc.scalar.activation(e[:], ps[:], EXP)
                        if fillers and (mi % 2 == 1 if not last_jn
                                        else mi == 3):
                            fillers.pop(0)()
                        if pend is not None:
                            attnv(*pend)
                        pend = (mi, e)
                    attnv(*pend)
                    for sub, po in ((0, po_e), (1, po_o)):
                        # recip_approx_fast NaNs on partition-offset inputs;
                        # stage the denominator row at partition 0 first.
                        dn = smp.tile([1, 512], F32, tag=f"dn{sub}",
                                      name=f"dn{sub}")
                        nc.vector.tensor_copy(dn[:], po[64:65, :])
                        rf = smp.tile([1, 512], F32, tag=f"rf{sub}",
                                      name=f"rf{sub}")
                        nc.vector.reciprocal_approx_fast(rf[:], dn[:])
                        pbs = smp.tile([64, 512], F32, tag=f"pbs{sub}",
                                       name=f"pbs{sub}")
                        nc.gpsimd.partition_broadcast(pbs[:], rf[:])
                        rsl = slice(sub * 64, sub * 64 + 64)
                        nc.vector.tensor_mul(aot[hp][rsl, nsl], po[0:64, :],
                                             pbs[:])

                # ---------------- main pipeline ----------------
                with nc.named_scope("attn"):
                    xs = qproj_load(0)
                    qproj_pair(0, 0, xs)
                    for jn in range(NJ):
                        fillers = []
                        if jn == 0:
                            xs0 = xs
                            for p in range(1, IT):
                                fillers += [
                                    (lambda p=p: kproj_half(p, 0)),
                                    (lambda p=p: kproj_half(p, 1)),
                                    (lambda p=p: qproj_pair(0, p, xs0))]
                        if jn + 1 < NJ:
                            xs = qproj_load(jn + 1)
                            fillers += [
                                (lambda p=p, xs=xs, j=jn + 1: qproj_pair(j, p, xs))
                                for p in range(IT)]
                        if jn >= 1:
                            fillers += [
                                (lambda nt=nt: oproj_tile(nt))
                                for nt in range(4 * (jn - 1), 4 * jn)]
                        for hp in range(IT):
                            attn_iter(hp, jn, fillers, jn == NJ - 1)
                        for f in fillers:
                            f()
                    for hp in range(IT):
                        oproj_tile(4 * (NJ - 1) + hp)
    nc.compile()
    return nc


_NC_CACHE = None


def kernel(x, context, Wq, Wk, Wv, Wo, bo, _trace=False):
    global _NC_CACHE, LAST_RESULTS
    x = np.asarray(x, np.float32)
    context = np.asarray(context, np.float32)
    scale = np.float32(DH ** -0.5)

    if _NC_CACHE is None:
        _NC_CACHE = build_nc()
    nc = _NC_CACHE

    Wq32 = np.asarray(Wq, np.float32)
    Wk32 = np.asarray(Wk, np.float32)
    Wv32 = np.asarray(Wv, np.float32)
    Wo32 = np.asarray(Wo, np.float32)
    in_maps = []
    for c in range(NC):
        b, g = c // 2, c % 2
        sl = slice(g * IS, (g + 1) * IS)
        m = {
            "xT": np.ascontiguousarray(x[b].T).astype(BF),
            "cT": np.ascontiguousarray(context[b].T).astype(BF),
            "wq": np.ascontiguousarray(Wq32[:, sl] * scale).astype(BF),
            "wk": np.ascontiguousarray(Wk32[:, sl]).astype(BF),
            "wv": np.ascontiguousarray(Wv32[:, sl]).astype(BF),
            "wo": np.ascontiguousarray(Wo32[sl, :]).astype(BF),
        }
        in_maps.append(m)
    res = run_bass_kernel_spmd(nc, in_maps, core_ids=list(range(NC)),
                               trace=_trace)
    LAST_RESULTS = res
    out = np.empty((B, N, QD), np.float32)
    bo32 = np.asarray(bo, np.float32)
    for b in range(B):
        out[b] = res.results[2 * b]["out"] + res.results[2 * b + 1]["out"] + bo32
    return out


# revision 15
# speedup vs baseline: 1.2727x; 1.0071x over previous
"""CrossAttention Trainium2 kernel, 8-core SPMD, bf16 matmuls, fp32 PSUM.

Sharding: core c -> (batch b = c//2, head-group g = c%2).  Each core computes
8 of the 16 heads for one batch: q/k/v projections restricted to its
inner-dim slice [g*512:(g+1)*512], full attention for those heads, and a
partial output projection (contraction over its 512 inner dims).  The host
sums the two partial outputs per batch and adds the bias.

Precision: every matmul runs on the PE in bf16 (1 cycle/row; separate
LDWEIGHTS instructions pipeline ahead of in-flight matmuls, unlike
fp32/fp32r whose 4-byte weight load serializes into the matmul), with fp32
PSUM accumulation.  Measured rel err ~3e-3 vs the 2e-2 budget.

Per-core dataflow (all intermediates SBUF-resident, no DRAM round-trips):
  vproj: v[m,512] -> va tiles [128, 8*65] (col h*65+64 is 1.0 so the
         softmax denominator rides the attn@v matmul)
  kproj: K^T head-pair tiles kt[p][128, M] (head 2p rows 0:64, 2p+1 64:128)
  qproj: Q^T head-pair tiles qt[p][128, N]  (scale folded into Wq)
  attn per (pair, jn-512-chunk): for each m-tile, two row-tiled K=64
    matmuls (array rows 0-63 / 64-127, concurrent) write simT into a
    2-bank PSUM tile; one wide exp (N=1024) -> bf16 e tile; two M=65
    attn@v matmuls accumulate [oT; denom] in per-head PSUM, software-
    pipelined one m-tile behind the exp so the PE never stalls on ACT.
    Normalize via reciprocal_approx_fast (input staged at partition 0 --
    the custom DVE op NaNs on partition-offset APs) + gpsimd
    partition_broadcast + DVE mul, written straight into bf16 aoT tiles.
  Only vproj + kproj(pair0) + qproj(jn0,pair0) run as a serial prefix;
  all remaining kproj/qproj chains and the oproj output tiles are fed as
  filler thunks between the attention matmuls, keeping the PE queue busy
  while the ACT engine streams the exp instructions.
"""
import sys

sys.path.insert(0, "/opt/trn_rl_repo")

import numpy as np
import ml_dtypes

BF = ml_dtypes.bfloat16

import concourse.bacc as bacc
import concourse.mybir as mybir
import concourse.tile as tile
from concourse.bass_utils import run_bass_kernel_spmd

# bass_utils imports antenv.axon_hooks when trace=True; the read-only antenv
# package in this image lacks it, so register a no-op stub if missing.
try:
    import antenv.axon_hooks  # noqa: F401
except ImportError:
    import types as _types

    _stub = _types.ModuleType("antenv.axon_hooks")
    _stub.get_axon_ntff_profile_hook = lambda: None
    _stub.set_axon_ntff_profile_hook = lambda h: None
    sys.modules["antenv.axon_hooks"] = _stub

F32 = mybir.dt.float32
BF16 = mybir.dt.bfloat16
EXP = mybir.ActivationFunctionType.Exp

B, N, M = 4, 2048, 1024
QD, CD = 1024, 768
HEADS, DH = 16, 64
INNER = HEADS * DH
HG = 8            # heads per core
IS = HG * DH      # inner slice per core = 512
NC = 8

KQ = QD // 128    # 8  K-tiles for q projection
KC = CD // 128    # 6  K-tiles for k/v projection
NJ = N // 512     # 4  n chunks
MT = M // 128     # 8  m tiles
IT = IS // 128    # 4  inner tiles (= head pairs)

LAST_RESULTS = None  # stashed BassKernelResults for test.py introspection


def build_nc():
    nc = bacc.Bacc("TRN2", target_bir_lowering=False, debug=False, num_devices=NC)

    def din(name, shape):
        return nc.dram_tensor(name, shape, BF16, kind="ExternalInput").ap()

    xT = din("xT", [QD, N])
    cT = din("cT", [CD, M])
    wq = din("wq", [QD, IS])
    wk = din("wk", [CD, IS])
    wv = din("wv", [CD, IS])
    wo = din("wo", [IS, QD])
    out = nc.dram_tensor("out", [N, QD], F32, kind="ExternalOutput").ap()

    with tile.TileContext(nc) as tc:
        with tc.tile_pool(name="pers", bufs=1) as pers, \
             tc.tile_pool(name="wp", bufs=1) as wp, \
             tc.tile_pool(name="ps", bufs=1, space="PSUM") as psp, \
             tc.tile_pool(name="po", bufs=1, space="PSUM") as pop:

            qt = [pers.tile([128, N], BF16, tag=f"qt{p}", name=f"qt{p}")
                  for p in range(IT)]
            kt = [pers.tile([128, M], BF16, tag=f"kt{p}", name=f"kt{p}")
                  for p in range(IT)]
            va = [pers.tile([128, HG * 65], BF16, tag=f"va{mi}", name=f"va{mi}")
                  for mi in range(MT)]
            aot = [pers.tile([128, N], BF16, tag=f"aot{p}", name=f"aot{p}")
                   for p in range(IT)]
            wq_sb = [wp.tile([128, IS], BF16, tag=f"wq{k}", name=f"wq{k}")
                     for k in range(KQ)]
            wo_sb = [wp.tile([128, QD], BF16, tag=f"wo{k}", name=f"wo{k}")
                     for k in range(IT)]

            # ---------------- k/v projections (cT loaded once) -----------
            with nc.named_scope("kvproj"):
                cp = wp
                ctb = cp.tile([128, KC * M], BF16, tag="ctb", name="ctb")
                ct_sb = [ctb[:, k * M:(k + 1) * M] for k in range(KC)]
                wk_sb = [cp.tile([128, IS], BF16, tag=f"wk{k}", name=f"wk{k}")
                         for k in range(KC)]
                wv_sb = [cp.tile([128, IS], BF16, tag=f"wv{k}", name=f"wv{k}")
                         for k in range(KC)]
                for k in range(KC):
                    ksl = slice(k * 128, (k + 1) * 128)
                    nc.sync.dma_start(wv_sb[k][:], wv[ksl, :])
                ctb3 = ctb[:].rearrange("p (k c) -> p k c", c=M)
                cT3 = cT[:, :].rearrange("(k p) c -> p k c", p=128)
                nc.sync.dma_start(ctb3[:, :, 0:512], cT3[:, :, 0:512])
                for k in range(KC):
                    ksl = slice(k * 128, (k + 1) * 128)
                    nc.sync.dma_start(wk_sb[k][:], wk[ksl, :])
                nc.sync.dma_start(ctb3[:, :, 512:M], cT3[:, :, 512:M])
                for k in range(KQ):
                    ksl = slice(k * 128, (k + 1) * 128)
                    nc.sync.dma_start(wq_sb[k][:], wq[ksl, :])

                # vproj: out v[m-tile, inner 512] -> va (col h*65+64 = 1.0)
                onesf = cp.tile([128, HG], F32, tag="onesf", name="onesf")
                nc.vector.memset(onesf[:], 1.0)
                for mi in range(MT):
                    msl = slice(mi * 128, (mi + 1) * 128)
                    ps = psp.tile([128, 512], F32, tag="mm", name="mm", bufs=2)
                    for k in range(KC):
                        nc.tensor.matmul(ps[:], ct_sb[k][:, msl], wv_sb[k][:],
                                         start=(k == 0), stop=(k == KC - 1))
                    vcol = va[mi][:].rearrange("p (h c) -> p h c", c=65)
                    psv = ps[:].rearrange("p (h c) -> p h c", c=64)
                    nc.vector.tensor_copy(vcol[:, :, 0:64], psv[:])
                    nc.vector.tensor_copy(vcol[:, :, 64], onesf[:])

                # kproj chain for one (pair, m-half); pair 0 runs in the
                # prefix, pairs 1-3 become attn fillers
                def kproj_half(p, jm):
                    msl = slice(jm * 512, (jm + 1) * 512)
                    isl = slice(p * 128, (p + 1) * 128)
                    ps = psp.tile([128, 512], F32, tag="mm", name="mm", bufs=2)
                    for k in range(KC):
                        nc.tensor.matmul(ps[:], wk_sb[k][:, isl],
                                         ct_sb[k][:, msl],
                                         start=(k == 0), stop=(k == KC - 1))
                    nc.vector.tensor_copy(kt[p][:, msl], ps[:])

                for jm in range(M // 512):
                    kproj_half(0, jm)
                # load wo while attention runs
                for k in range(IT):
                    ksl = slice(k * 128, (k + 1) * 128)
                    nc.sync.dma_start(wo_sb[k][:], wo[ksl, :])

            # ---------------- qproj helper (per n-chunk) ----------------
            with tc.tile_pool(name="xs", bufs=2) as xsp, \
                 tc.tile_pool(name="sm", bufs=1) as smp, \
                 tc.tile_pool(name="ob", bufs=2) as obp:

                def qproj_load(jn):
                    nsl = slice(jn * 512, (jn + 1) * 512)
                    xs = [xsp.tile([128, 512], BF16, tag=f"x{k}", name=f"x{k}")
                          for k in range(KQ)]
                    for k in range(KQ):
                        ksl = slice(k * 128, (k + 1) * 128)
                        nc.sync.dma_start(xs[k][:], xT[ksl, nsl])
                    return xs

                def qproj_pair(jn, p, xs):
                    nsl = slice(jn * 512, (jn + 1) * 512)
                    isl = slice(p * 128, (p + 1) * 128)
                    ps = psp.tile([128, 512], F32, tag="mm", name="mm", bufs=2)
                    for k in range(KQ):
                        nc.tensor.matmul(ps[:], wq_sb[k][:, isl], xs[k][:],
                                         start=(k == 0), stop=(k == KQ - 1))
                    nc.vector.tensor_copy(qt[p][:, nsl], ps[:])

                def oproj_tile(nt):
                    tsl = slice(nt * 128, (nt + 1) * 128)
                    ob = obp.tile([128, QD], F32, tag="ob", name="ob")
                    for half in range(QD // 512):
                        qsl = slice(half * 512, (half + 1) * 512)
                        ps = psp.tile([128, 512], F32, tag="mm", name="mm",
                                      bufs=2)
                        for k in range(IT):
                            nc.tensor.matmul(ps[:], aot[k][:, tsl],
                                             wo_sb[k][:, qsl],
                                             start=(k == 0), stop=(k == IT - 1))
                        nc.vector.tensor_copy(ob[:, qsl], ps[:])
                    nc.sync.dma_start(out[tsl, :], ob[:])

                def attn_iter(hp, jn, fillers, last_jn=False):
                    nsl = slice(jn * 512, (jn + 1) * 512)
                    he, ho = 2 * hp, 2 * hp + 1
                    po_e = pop.tile([65, 512], F32, tag="poe", name="poe")
                    po_o = pop.tile([65, 512], F32, tag="poo", name="poo")

                    def attnv(mi, e):
                        nc.tensor.matmul(po_e[:], va[mi][:, he * 65:he * 65 + 65],
                                         e[:, 0:512], start=(mi == 0),
                                         stop=(mi == MT - 1),
                                         skip_group_check=True)
                        nc.tensor.matmul(po_o[:], va[mi][:, ho * 65:ho * 65 + 65],
                                         e[:, 512:1024], start=(mi == 0),
                                         stop=(mi == MT - 1),
                                         skip_group_check=True)

                    # attnv runs one m-tile behind sim/exp so the PE never
                    # stalls on the ACT exp; filler matmul chains (qproj /
                    # oproj) slot in between to keep the PE queue fed.
                    pend = None
                    for mi in range(MT):
                        msl = slice(mi * 128, (mi + 1) * 128)
                        ps = psp.tile([128, 1024], F32, tag="sp", name="sp",
                                      bufs=2)
                        nc.tensor.matmul(ps[:, 0:512], kt[hp][0:64, msl],
                                         qt[hp][0:64, nsl], start=True,
                                         stop=True)
                        nc.tensor.matmul(ps[:, 512:1024], kt[hp][64:128, msl],
                                         qt[hp][64:128, nsl], start=True,
                                         stop=True)
                        e = smp.tile([128, 1024], BF16, tag="e", name="e",
                                     bufs=4)
                        nc.scalar.activation(e[:], ps[:], EXP)
                        if fillers and (mi % 2 == 1 if not last_jn
                                        else mi == 3):
                            fillers.pop(0)()
                        if pend is not None:
                            attnv(*pend)
                        pend = (mi, e)
                    attnv(*pend)
                    for sub, po in ((0, po_e), (1, po_o)):
                        # recip_approx_fast NaNs on partition-offset inputs;
                        # stage the denominator row at partition 0 first.
                        dn = smp.tile([1, 512], F32, tag=f"dn{sub}",
                                      name=f"dn{sub}")
                        nc.vector.tensor_copy(dn[:], po[64:65, :])
                        rf = smp.tile([1, 512], F32, tag=f"rf{sub}",
                                      name=f"rf{sub}")
                        nc.vector.reciprocal_approx_fast(rf[:], dn[:])
                        pbs = smp.tile([64, 512], F32, tag=f"pbs{sub}",
                                       name=f"pbs{sub}")
                        nc.gpsimd.partition_broadcast(pbs[:], rf[:])
                        rsl = slice(sub * 64, sub * 64 + 64)
                        nc.vector.tensor_mul(aot[hp][rsl, nsl], po[0:64, :],
                                             pbs[:])

                # ---------------- main pipeline ----------------
                with nc.named_scope("attn"):
                    xs = qproj_load(0)
                    qproj_pair(0, 0, xs)
                    for jn in range(NJ):
                        fillers = []
                        if jn == 0:
                            xs0 = xs
                            for p in range(1, IT):
                                fillers += [
                                    (lambda p=p: kproj_half(p, 0)),
                                    (lambda p=p: kproj_half(p, 1)),
                                    (lambda p=p: qproj_pair(0, p, xs0))]
                        if jn + 1 < NJ:
                            xs = qproj_load(jn + 1)
                            fillers += [
                                (lambda p=p, xs=xs, j=jn + 1: qproj_pair(j, p, xs))
                                for p in range(IT)]
                        if jn >= 1:
                            fillers += [
                                (lambda nt=nt: oproj_tile(nt))
                                for nt in range(4 * (jn - 1), 4 * jn)]
                        for hp in range(IT):
                            attn_iter(hp, jn, fillers, jn == NJ - 1)
                        for f in fillers:
                            f()
                    for hp in range(IT):
                        oproj_tile(4 * (NJ - 1) + hp)
    nc.compile()
    return nc


_NC_CACHE = None


def kernel(x, context, Wq, Wk, Wv, Wo, bo, _trace=False):
    global _NC_CACHE, LAST_RESULTS
    x = np.asarray(x, np.float32)
    context = np.asarray(context, np.float32)
    scale = np.float32(DH ** -0.5)

    if _NC_CACHE is None:
        _NC_CACHE = build_nc()
    nc = _NC_CACHE

    Wq32 = np.asarray(Wq, np.float32)
    Wk32 = np.asarray(Wk, np.float32)
    Wv32 = np.asarray(Wv, np.float32)
    Wo32 = np.asarray(Wo, np.float32)
    in_maps = []
    for c in range(NC):
        b, g = c // 2, c % 2
        sl = slice(g * IS, (g + 1) * IS)
        m = {
            "xT": np.ascontiguousarray(x[b].T).astype(BF),
            "cT": np.ascontiguousarray(context[b].T).astype(BF),
            "wq": np.ascontiguousarray(Wq32[:, sl] * scale).astype(BF),
            "wk": np.ascontiguousarray(Wk32[:, sl]).astype(BF),
            "wv": np.ascontiguousarray(Wv32[:, sl]).astype(BF),
            "wo": np.ascontiguousarray(Wo32[sl, :]).astype(BF),
        }
        in_maps.append(m)
    res = run_bass_kernel_spmd(nc, in_maps, core_ids=list(range(NC)),
                               trace=_trace)
    LAST_RESULTS = res
    out = np.empty((B, N, QD), np.float32)
    bo32 = np.asarray(bo, np.float32)
    for b in range(B):
        out[b] = res.results[2 * b]["out"] + res.results[2 * b + 1]["out"] + bo32
    return out


# revision 16
# speedup vs baseline: 1.2935x; 1.0163x over previous
"""CrossAttention Trainium2 kernel, 8-core SPMD, bf16 matmuls, fp32 PSUM.

Sharding: core c -> (batch b = c//2, head-group g = c%2).  Each core computes
8 of the 16 heads for one batch: q/k/v projections restricted to its
inner-dim slice [g*512:(g+1)*512], full attention for those heads, and a
partial output projection (contraction over its 512 inner dims).  The host
sums the two partial outputs per batch and adds the bias.

Precision: every matmul runs on the PE in bf16 (1 cycle/row; separate
LDWEIGHTS instructions pipeline ahead of in-flight matmuls, unlike
fp32/fp32r whose 4-byte weight load serializes into the matmul), with fp32
PSUM accumulation.  Measured rel err ~3e-3 vs the 2e-2 budget.

Per-core dataflow (all intermediates SBUF-resident, no DRAM round-trips):
  vproj: v[m,512] -> va tiles [128, 8*65] (col h*65+64 is 1.0 so the
         softmax denominator rides the attn@v matmul)
  kproj: K^T head-pair tiles kt[p][128, M] (head 2p rows 0:64, 2p+1 64:128)
  qproj: Q^T head-pair tiles qt[p][128, N]  (scale folded into Wq)
  attn per (pair, jn-512-chunk): for each m-tile, two row-tiled K=64
    matmuls (array rows 0-63 / 64-127, concurrent) write simT into a
    2-bank PSUM tile; one wide exp (N=1024) -> bf16 e tile; two M=65
    attn@v matmuls accumulate [oT; denom] in per-head PSUM, software-
    pipelined one m-tile behind the exp so the PE never stalls on ACT.
    Normalize via reciprocal_approx_fast (input staged at partition 0 --
    the custom DVE op NaNs on partition-offset APs) + gpsimd
    partition_broadcast + DVE mul, written straight into bf16 aoT tiles.
  Only vproj + kproj(pair0) + qproj(jn0,pair0) run as a serial prefix;
  all remaining kproj/qproj chains and the oproj output tiles are fed as
  filler thunks between the attention matmuls, keeping the PE queue busy
  while the ACT engine streams the exp instructions.
"""
import sys

sys.path.insert(0, "/opt/trn_rl_repo")

import numpy as np
import ml_dtypes

BF = ml_dtypes.bfloat16

import concourse.bacc as bacc
import concourse.mybir as mybir
import concourse.tile as tile
from concourse.bass_utils import run_bass_kernel_spmd

# bass_utils imports antenv.axon_hooks when trace=True; the read-only antenv
# package in this image lacks it, so register a no-op stub if missing.
try:
    import antenv.axon_hooks  # noqa: F401
except ImportError:
    import types as _types

    _stub = _types.ModuleType("antenv.axon_hooks")
    _stub.get_axon_ntff_profile_hook = lambda: None
    _stub.set_axon_ntff_profile_hook = lambda h: None
    sys.modules["antenv.axon_hooks"] = _stub

F32 = mybir.dt.float32
BF16 = mybir.dt.bfloat16
EXP = mybir.ActivationFunctionType.Exp

B, N, M = 4, 2048, 1024
QD, CD = 1024, 768
HEADS, DH = 16, 64
INNER = HEADS * DH
HG = 8            # heads per core
IS = HG * DH      # inner slice per core = 512
NC = 8

KQ = QD // 128    # 8  K-tiles for q projection
KC = CD // 128    # 6  K-tiles for k/v projection
NJ = N // 512     # 4  n chunks
MT = M // 128     # 8  m tiles
IT = IS // 128    # 4  inner tiles (= head pairs)

LAST_RESULTS = None  # stashed BassKernelResults for test.py introspection


def build_nc():
    nc = bacc.Bacc("TRN2", target_bir_lowering=False, debug=False, num_devices=NC)

    def din(name, shape):
        return nc.dram_tensor(name, shape, BF16, kind="ExternalInput").ap()

    xT = din("xT", [QD, N])
    cT = din("cT", [CD, M])
    wq = din("wq", [QD, IS])
    wk = din("wk", [CD, IS])
    wv = din("wv", [CD, IS])
    wo = din("wo", [IS, QD])
    out = nc.dram_tensor("out", [N, QD], F32, kind="ExternalOutput").ap()

    with tile.TileContext(nc) as tc:
        with tc.tile_pool(name="pers", bufs=1) as pers, \
             tc.tile_pool(name="wp", bufs=1) as wp, \
             tc.tile_pool(name="ps", bufs=1, space="PSUM") as psp, \
             tc.tile_pool(name="po", bufs=1, space="PSUM") as pop:

            qt = [pers.tile([128, N], BF16, tag=f"qt{p}", name=f"qt{p}")
                  for p in range(IT)]
            kt = [pers.tile([128, M], BF16, tag=f"kt{p}", name=f"kt{p}")
                  for p in range(IT)]
            va = [pers.tile([128, HG * 65], BF16, tag=f"va{mi}", name=f"va{mi}")
                  for mi in range(MT)]
            aot = [pers.tile([128, N], BF16, tag=f"aot{p}", name=f"aot{p}")
                   for p in range(IT)]
            wqb = wp.tile([128, KQ * IS], BF16, tag="wqb", name="wqb")
            wq_sb = [wqb[:, k * IS:(k + 1) * IS] for k in range(KQ)]
            wo_sb = [wp.tile([128, QD], BF16, tag=f"wo{k}", name=f"wo{k}")
                     for k in range(IT)]

            # ---------------- k/v projections (cT loaded once) -----------
            with nc.named_scope("kvproj"):
                cp = wp
                ctb = cp.tile([128, KC * M], BF16, tag="ctb", name="ctb")
                ct_sb = [ctb[:, k * M:(k + 1) * M] for k in range(KC)]
                wkb = cp.tile([128, KC * IS], BF16, tag="wkb", name="wkb")
                wk_sb = [wkb[:, k * IS:(k + 1) * IS] for k in range(KC)]
                wvb = cp.tile([128, KC * IS], BF16, tag="wvb", name="wvb")
                wv_sb = [wvb[:, k * IS:(k + 1) * IS] for k in range(KC)]
                nc.sync.dma_start(
                    wvb[:].rearrange("p (k c) -> p k c", c=IS),
                    wv[:, :].rearrange("(k p) c -> p k c", p=128))
                ctb3 = ctb[:].rearrange("p (k c) -> p k c", c=M)
                cT3 = cT[:, :].rearrange("(k p) c -> p k c", p=128)
                nc.sync.dma_start(ctb3[:, :, 0:512], cT3[:, :, 0:512])
                nc.sync.dma_start(
                    wkb[:].rearrange("p (k c) -> p k c", c=IS),
                    wk[:, :].rearrange("(k p) c -> p k c", p=128))
                nc.sync.dma_start(ctb3[:, :, 512:M], cT3[:, :, 512:M])
                nc.sync.dma_start(
                    wqb[:].rearrange("p (k c) -> p k c", c=IS),
                    wq[:, :].rearrange("(k p) c -> p k c", p=128))

                # vproj: out v[m-tile, inner 512] -> va (col h*65+64 = 1.0)
                onesf = cp.tile([128, HG], F32, tag="onesf", name="onesf")
                nc.vector.memset(onesf[:], 1.0)
                for mi in range(MT):
                    msl = slice(mi * 128, (mi + 1) * 128)
                    ps = psp.tile([128, 512], F32, tag="mm", name="mm", bufs=2)
                    for k in range(KC):
                        nc.tensor.matmul(ps[:], ct_sb[k][:, msl], wv_sb[k][:],
                                         start=(k == 0), stop=(k == KC - 1))
                    vcol = va[mi][:].rearrange("p (h c) -> p h c", c=65)
                    psv = ps[:].rearrange("p (h c) -> p h c", c=64)
                    nc.vector.tensor_copy(vcol[:, :, 0:64], psv[:])
                    nc.vector.tensor_copy(vcol[:, :, 64], onesf[:])

                # kproj chain for one (pair, m-half); pair 0 runs in the
                # prefix, pairs 1-3 become attn fillers
                def kproj_half(p, jm):
                    msl = slice(jm * 512, (jm + 1) * 512)
                    isl = slice(p * 128, (p + 1) * 128)
                    ps = psp.tile([128, 512], F32, tag="mm", name="mm", bufs=2)
                    for k in range(KC):
                        nc.tensor.matmul(ps[:], wk_sb[k][:, isl],
                                         ct_sb[k][:, msl],
                                         start=(k == 0), stop=(k == KC - 1))
                    nc.vector.tensor_copy(kt[p][:, msl], ps[:])

                for jm in range(M // 512):
                    kproj_half(0, jm)
                # load wo while attention runs
                for k in range(IT):
                    ksl = slice(k * 128, (k + 1) * 128)
                    nc.sync.dma_start(wo_sb[k][:], wo[ksl, :])

            # ---------------- qproj helper (per n-chunk) ----------------
            with tc.tile_pool(name="xs", bufs=2) as xsp, \
                 tc.tile_pool(name="sm", bufs=1) as smp, \
                 tc.tile_pool(name="ob", bufs=2) as obp:

                def qproj_load(jn):
                    nsl = slice(jn * 512, (jn + 1) * 512)
                    xs = [xsp.tile([128, 512], BF16, tag=f"x{k}", name=f"x{k}")
                          for k in range(KQ)]
                    for k in range(KQ):
                        ksl = slice(k * 128, (k + 1) * 128)
                        nc.sync.dma_start(xs[k][:], xT[ksl, nsl])
                    return xs

                def qproj_pair(jn, p, xs):
                    nsl = slice(jn * 512, (jn + 1) * 512)
                    isl = slice(p * 128, (p + 1) * 128)
                    ps = psp.tile([128, 512], F32, tag="mm", name="mm", bufs=2)
                    for k in range(KQ):
                        nc.tensor.matmul(ps[:], wq_sb[k][:, isl], xs[k][:],
                                         start=(k == 0), stop=(k == KQ - 1))
                    nc.vector.tensor_copy(qt[p][:, nsl], ps[:])

                def oproj_tile(nt):
                    tsl = slice(nt * 128, (nt + 1) * 128)
                    ob = obp.tile([128, QD], F32, tag="ob", name="ob")
                    for half in range(QD // 512):
                        qsl = slice(half * 512, (half + 1) * 512)
                        ps = psp.tile([128, 512], F32, tag="mm", name="mm",
                                      bufs=2)
                        for k in range(IT):
                            nc.tensor.matmul(ps[:], aot[k][:, tsl],
                                             wo_sb[k][:, qsl],
                                             start=(k == 0), stop=(k == IT - 1))
                        nc.vector.tensor_copy(ob[:, qsl], ps[:])
                    nc.sync.dma_start(out[tsl, :], ob[:])

                def attn_iter(hp, jn, fillers, last_jn=False):
                    nsl = slice(jn * 512, (jn + 1) * 512)
                    he, ho = 2 * hp, 2 * hp + 1
                    po_e = pop.tile([65, 512], F32, tag="poe", name="poe")
                    po_o = pop.tile([65, 512], F32, tag="poo", name="poo")

                    def attnv(mi, e):
                        nc.tensor.matmul(po_e[:], va[mi][:, he * 65:he * 65 + 65],
                                         e[:, 0:512], start=(mi == 0),
                                         stop=(mi == MT - 1),
                                         skip_group_check=True)
                        nc.tensor.matmul(po_o[:], va[mi][:, ho * 65:ho * 65 + 65],
                                         e[:, 512:1024], start=(mi == 0),
                                         stop=(mi == MT - 1),
                                         skip_group_check=True)

                    # attnv runs one m-tile behind sim/exp so the PE never
                    # stalls on the ACT exp; filler matmul chains (qproj /
                    # oproj) slot in between to keep the PE queue fed.
                    pend = None
                    for mi in range(MT):
                        msl = slice(mi * 128, (mi + 1) * 128)
                        ps = psp.tile([128, 1024], F32, tag="sp", name="sp",
                                      bufs=2)
                        nc.tensor.matmul(ps[:, 0:512], kt[hp][0:64, msl],
                                         qt[hp][0:64, nsl], start=True,
                                         stop=True)
                        nc.tensor.matmul(ps[:, 512:1024], kt[hp][64:128, msl],
                                         qt[hp][64:128, nsl], start=True,
                                         stop=True)
                        e = smp.tile([128, 1024], BF16, tag="e", name="e",
                                     bufs=4)
                        nc.scalar.activation(e[:], ps[:], EXP)
                        if fillers and (mi % 2 == 1 if not last_jn
                                        else mi == 3):
                            fillers.pop(0)()
                        if pend is not None:
                            attnv(*pend)
                        pend = (mi, e)
                    attnv(*pend)
                    for sub, po in ((0, po_e), (1, po_o)):
                        # recip_approx_fast NaNs on partition-offset inputs;
                        # stage the denominator row at partition 0 first.
                        dn = smp.tile([1, 512], F32, tag=f"dn{sub}",
                                      name=f"dn{sub}")
                        nc.vector.tensor_copy(dn[:], po[64:65, :])
                        rf = smp.tile([1, 512], F32, tag=f"rf{sub}",
                                      name=f"rf{sub}")
                        nc.vector.reciprocal_approx_fast(rf[:], dn[:])
                        pbs = smp.tile([64, 512], F32, tag=f"pbs{sub}",
                                       name=f"pbs{sub}")
                        nc.gpsimd.partition_broadcast(pbs[:], rf[:])
                        rsl = slice(sub * 64, sub * 64 + 64)
                        nc.vector.tensor_mul(aot[hp][rsl, nsl], po[0:64, :],
                                             pbs[:])

                # ---------------- main pipeline ----------------
                with nc.named_scope("attn"):
                    xs = qproj_load(0)
                    qproj_pair(0, 0, xs)
                    for jn in range(NJ):
                        fillers = []
                        if jn == 0:
                            xs0 = xs
                            for p in range(1, IT):
                                fillers += [
                                    (lambda p=p: kproj_half(p, 0)),
                                    (lambda p=p: kproj_half(p, 1)),
                                    (lambda p=p: qproj_pair(0, p, xs0))]
                        if jn + 1 < NJ:
                            xs = qproj_load(jn + 1)
                            fillers += [
                                (lambda p=p, xs=xs, j=jn + 1: qproj_pair(j, p, xs))
                                for p in range(IT)]
                        if jn >= 1:
                            fillers += [
                                (lambda nt=nt: oproj_tile(nt))
                                for nt in range(4 * (jn - 1), 4 * jn)]
                        for hp in range(IT):
                            attn_iter(hp, jn, fillers, jn == NJ - 1)
                        for f in fillers:
                            f()
                    for hp in range(IT):
                        oproj_tile(4 * (NJ - 1) + hp)
    nc.compile()
    return nc


_NC_CACHE = None


def kernel(x, context, Wq, Wk, Wv, Wo, bo, _trace=False):
    global _NC_CACHE, LAST_RESULTS
    x = np.asarray(x, np.float32)
    context = np.asarray(context, np.float32)
    scale = np.float32(DH ** -0.5)

    if _NC_CACHE is None:
        _NC_CACHE = build_nc()
    nc = _NC_CACHE

    Wq32 = np.asarray(Wq, np.float32)
    Wk32 = np.asarray(Wk, np.float32)
    Wv32 = np.asarray(Wv, np.float32)
    Wo32 = np.asarray(Wo, np.float32)
    in_maps = []
    for c in range(NC):
        b, g = c // 2, c % 2
        sl = slice(g * IS, (g + 1) * IS)
        m = {
            "xT": np.ascontiguousarray(x[b].T).astype(BF),
            "cT": np.ascontiguousarray(context[b].T).astype(BF),
            "wq": np.ascontiguousarray(Wq32[:, sl] * scale).astype(BF),
            "wk": np.ascontiguousarray(Wk32[:, sl]).astype(BF),
            "wv": np.ascontiguousarray(Wv32[:, sl]).astype(BF),
            "wo": np.ascontiguousarray(Wo32[sl, :]).astype(BF),
        }
        in_maps.append(m)
    res = run_bass_kernel_spmd(nc, in_maps, core_ids=list(range(NC)),
                               trace=_trace)
    LAST_RESULTS = res
    out = np.empty((B, N, QD), np.float32)
    bo32 = np.asarray(bo, np.float32)
    for b in range(B):
        out[b] = res.results[2 * b]["out"] + res.results[2 * b + 1]["out"] + bo32
    return out
